# revision 1
# baseline (speedup 1.0000x reference)
"""ChildSum TreeGRU on 8 Trainium2 NeuronCores.

Data-parallel over trees (16/core). fp16 on-chip; feature dim folded as
[128 partitions, 2 k-halves, cols]; within each tree level, nodes are stored
in bit-reversed order so children of the (bit-reversed-ordered) parents form
two contiguous blocks [left | right] -> every DVE op is packed/contiguous and
runs in the 2-byte 2x mode. GpSimd takes the z-sum path; ACT ops are fused
across both feature halves (2048 cols) when all biases are zero.

Levels 10(leaf)..6 run per group of 4 trees (wavefront across 4 groups);
level-6 results land in a joint buffer holding levels 0..6 for all 16 trees;
levels 5..0 are processed jointly. Host pre-permutes x and post-permutes the
output (bit-reversal + layout), which is free w.r.t. HW exec time.
"""
import sys

for p in ("/opt/trn_rl_repo", "/root/.axon_site/_ro/trn_rl_repo"):
    if p not in sys.path:
        sys.path.insert(0, p)

import numpy as np
import concourse.tile as tile
from concourse import bacc, mybir
from concourse.bass_utils import run_bass_kernel_spmd

f32 = mybir.dt.float32
fp16 = mybir.dt.float16
AF = mybir.ActivationFunctionType
ALU = mybir.AluOpType

T, DEPTH, NN, H = 128, 11, 2047, 256
NCORES = 8
TPC = T // NCORES          # 16 trees per core
G = 4                      # trees per group
NG = TPC // G              # 4 groups
NLEAF = 1 << (DEPTH - 1)   # 1024

CH_MM = 512                # matmul / psum chunk (one psum bank per half)
CH_V = 1024                # DVE chunk (free size 2048)
CH_G = 512                 # gpsimd chunk
GPS_MIN = 512              # use gpsimd for zs1/zhs when C >= this

# h_out column blocks (device layout, fp16). Group levels 10..7 then jbuf.
OFF10 = 0
OFF9 = OFF10 + TPC * 1024
OFF8 = OFF9 + TPC * 512
OFF7 = OFF8 + TPC * 256
OFFJ = OFF7 + TPC * 128            # 30720
JN = 127                            # nodes/tree in jbuf (levels 0..6)
OUTCOLS = OFFJ + JN * TPC           # 32752
# jbuf block start (in slots) for level l: levels 6,5,...,0 packed high->low
JOFF = {l: (JN - ((1 << (l + 1)) - 1)) for l in range(7)}


def _sig(l):
    s = np.zeros(1, np.int64)
    for _ in range(l):
        s = np.concatenate([2 * s, 2 * s + 1])
    return s


SIG = {l: _sig(l) for l in range(DEPTH)}


def _v3(t):
    """[128, 2*N] tile -> [128, 2, N] view."""
    return t[:].rearrange("p (a b) -> p a b", a=2)


def _gate(nc, P, wt3, rhs3, cols, out_sl, func, bias2, nm, rh_into=None, r_pool=None):
    """out = func(U @ rhs + b) over `cols` parent/child columns.

    rhs3: [128, 2, cols] fp16 view. out_sl(c0, n) -> output AP slice.
    If rh_into is given (r-gate), each chunk's result is multiplied into
    rh_into (in-place rh = r * hs) right after its activation.
    """
    for c0 in range(0, cols, CH_MM):
        n = min(CH_MM, cols - c0)
        ps = P["ps"].tile([128, 2, CH_MM], f32, name=f"ps{nm}_{c0}", tag="ps")
        for m in range(2):
            for k in range(2):
                nc.tensor.matmul(ps[:, m, :n], wt3[:, k, m * 128:(m + 1) * 128],
                                 rhs3[:, k, c0:c0 + n], start=(k == 0), stop=(k == 1))
        if rh_into is None:
            if bias2 is None:
                nc.scalar.activation(out_sl(c0, n), ps[:, :, :n], func)
            else:
                for m in range(2):
                    nc.scalar.activation(out_sl(c0, n, m), ps[:, m, :n], func,
                                         bias=bias2[m][:])
        else:
            rc = r_pool.tile([128, 2, CH_MM], fp16, name=f"r{nm}_{c0}", tag="r")
            if bias2 is None:
                nc.scalar.activation(rc[:, :, :n], ps[:, :, :n], func)
            else:
                for m in range(2):
                    nc.scalar.activation(rc[:, m, :n], ps[:, m, :n], func,
                                         bias=bias2[m][:])
            # rh = r * hs (in place into hs chunk)
            nc.vector.tensor_tensor(rh_into[:, :, c0:c0 + n], rc[:, :, :n],
                                    rh_into[:, :, c0:c0 + n], ALU.mult)


def _emit_level(nc, P, nm, NT, lv, hc3, out_sl, out_max_chunk, Wt, bias, fuse):
    """One GRU level: children hc3 [128, 2, 2C] -> parents via out_sl."""
    C = NT * (1 << lv)
    bz = None if fuse else bias["bz"]
    br = None if fuse else bias["br"]
    bc = None if fuse else bias["bc"]

    # hs = left + right children (packed 2x DVE)
    hs = P["hs"].tile([128, 2, C], fp16, name=f"hs{nm}", tag="hs")
    for c0 in range(0, C, CH_V):
        n = min(CH_V, C - c0)
        nc.vector.tensor_tensor(hs[:, :, c0:c0 + n], hc3[:, :, c0:c0 + n],
                                hc3[:, :, C + c0:C + c0 + n], ALU.add)

    # z = sigmoid(Uz @ hc + bz) over all 2C children
    z = P["z"].tile([128, 2, 2 * C], fp16, name=f"z{nm}", tag="z")
    _gate(nc, P, Wt["uz"], hc3, 2 * C,
          (lambda c0, n, m=None: z[:, :, c0:c0 + n] if m is None
           else z[:, m, c0:c0 + n]),
          AF.Sigmoid, bz, f"z{nm}")

    # zs1 = z_l + z_r (gpsimd TT when big), then -1 in place (DVE 4x)
    zs1 = P["zs1"].tile([128, 2, C], fp16, name=f"zs1{nm}", tag="zs1")
    eng_s = nc.gpsimd if C >= GPS_MIN else nc.vector
    for c0 in range(0, C, CH_G):
        n = min(CH_G, C - c0)
        eng_s.tensor_tensor(zs1[:, :, c0:c0 + n], z[:, :, c0:c0 + n],
                            z[:, :, C + c0:C + c0 + n], ALU.add)
    for c0 in range(0, C, CH_V):
        n = min(CH_V, C - c0)
        nc.vector.tensor_scalar(zs1[:, :, c0:c0 + n], zs1[:, :, c0:c0 + n],
                                -1.0, None, ALU.add)

    # r chain: r = sigmoid(Ur @ hs + br); rh = r * hs in place per chunk
    _gate(nc, P, Wt["ur"], hs[:, :, :], C, None, AF.Sigmoid, br, f"r{nm}",
          rh_into=hs, r_pool=P["r"])

    # h_cand = tanh(Uc @ rh + bc)
    hcand = P["hc"].tile([128, 2, C], fp16, name=f"hc{nm}", tag="hc")
    _gate(nc, P, Wt["uc"], hs[:, :, :], C,
          (lambda c0, n, m=None: hcand[:, :, c0:c0 + n] if m is None
           else hcand[:, m, c0:c0 + n]),
          AF.Tanh, bc, f"c{nm}")

    # zh = z * hc in place (DVE 2x)
    for c0 in range(0, 2 * C, CH_V):
        n = min(CH_V, 2 * C - c0)
        nc.vector.tensor_tensor(z[:, :, c0:c0 + n], z[:, :, c0:c0 + n],
                                hc3[:, :, c0:c0 + n], ALU.mult)

    # zhs = zh_l + zh_r into hs slot (gpsimd when big; hs free after c-matmul)
    eng_a = nc.gpsimd if C >= GPS_MIN else nc.vector
    for c0 in range(0, C, CH_G):
        n = min(CH_G, C - c0)
        eng_a.tensor_tensor(hs[:, :, c0:c0 + n], z[:, :, c0:c0 + n],
                            z[:, :, C + c0:C + c0 + n], ALU.add)

    # t = zs1 * hcand (in place); h_new = zhs - t
    for c0 in range(0, C, CH_V):
        n = min(CH_V, C - c0)
        nc.vector.tensor_tensor(hcand[:, :, c0:c0 + n], zs1[:, :, c0:c0 + n],
                                hcand[:, :, c0:c0 + n], ALU.mult)
    step = min(out_max_chunk, CH_V)
    for c0 in range(0, C, step):
        n = min(step, C - c0)
        nc.vector.tensor_tensor(out_sl(c0, n), hs[:, :, c0:c0 + n],
                                hcand[:, :, c0:c0 + n], ALU.subtract)


def _build(fuse):
    nc = bacc.Bacc("TRN2", debug=False)

    xT_d = nc.dram_tensor("xT", [H, TPC * NLEAF], fp16, kind="ExternalInput")
    wd = {}
    for gnm in ("w", "uz", "ur", "uc"):
        wd[gnm] = nc.dram_tensor(f"{gnm}T", [H, H], fp16, kind="ExternalInput")
    bd = {}
    if not fuse:
        for bnm in ("bw", "bz", "br", "bc"):
            bd[bnm] = nc.dram_tensor(bnm, [H, 1], f32, kind="ExternalInput")
    hout_d = nc.dram_tensor("h_out", [H, OUTCOLS], fp16, kind="ExternalOutput")

    with tile.TileContext(nc) as tc:
        from contextlib import ExitStack
        with ExitStack() as ctx:
            P = {}
            P["const"] = ctx.enter_context(tc.tile_pool(name="const", bufs=1))
            P["xg"] = ctx.enter_context(tc.tile_pool(name="xg", bufs=4))
            P["h10"] = ctx.enter_context(tc.tile_pool(name="h10", bufs=2))
            P["h9"] = ctx.enter_context(tc.tile_pool(name="h9", bufs=2))
            P["h8"] = ctx.enter_context(tc.tile_pool(name="h8", bufs=2))
            P["h7"] = ctx.enter_context(tc.tile_pool(name="h7", bufs=2))
            P["jbuf"] = ctx.enter_context(tc.tile_pool(name="jbuf", bufs=1))
            P["hs"] = ctx.enter_context(tc.tile_pool(name="hs", bufs=2))
            P["r"] = ctx.enter_context(tc.tile_pool(name="r", bufs=4))
            P["hc"] = ctx.enter_context(tc.tile_pool(name="hc", bufs=2))
            P["z"] = ctx.enter_context(tc.tile_pool(name="z", bufs=2))
            P["zs1"] = ctx.enter_context(tc.tile_pool(name="zs1", bufs=2))
            P["ps"] = ctx.enter_context(tc.tile_pool(name="ps", bufs=4, space="PSUM"))

            cp = P["const"]
            Wt = {}
            for gnm in ("w", "uz", "ur", "uc"):
                wtile = cp.tile([128, 2, H], fp16, name=f"{gnm}T")
                Wt[gnm] = wtile
                for k in range(2):
                    nc.sync.dma_start(wtile[:, k, :],
                                      wd[gnm].ap()[k * 128:(k + 1) * 128, :])
            bias = {}
            if not fuse:
                for bnm in ("bw", "bz", "br", "bc"):
                    bias[bnm] = [cp.tile([128, 1], f32, name=f"{bnm}{m}")
                                 for m in range(2)]
                    for m in range(2):
                        nc.sync.dma_start(bias[bnm][m][:],
                                          bd[bnm].ap()[m * 128:(m + 1) * 128, :])
            bw2 = None if fuse else bias["bw"]

            jb = P["jbuf"].tile([128, 2, JN * TPC], fp16, name="jbuf")

            def jview(l):
                a = JOFF[l] * TPC
                return jb[:, :, a:a + (1 << l) * TPC]

            # --- group phase -------------------------------------------------
            def load_x(g):
                ts = []
                for q in range(2):
                    xt = P["xg"].tile([128, 2, 2048], fp16,
                                      name=f"x{g}_{q}", tag="xg")
                    for k in range(2):
                        c0 = g * G * NLEAF + q * 2048
                        nc.sync.dma_start(xt[:, k, :],
                                          xT_d.ap()[k * 128:(k + 1) * 128,
                                                    c0:c0 + 2048])
                    ts.append(xt)
                return ts

            def emit_leaf(g, xts):
                h10 = P["h10"].tile([128, 2, G * NLEAF], fp16,
                                    name=f"h10g{g}", tag="h10")
                for q in range(2):
                    _gate(nc, P, Wt["w"], xts[q][:, :, :], 2048,
                          (lambda c0, n, m=None, q=q:
                           h10[:, :, q * 2048 + c0:q * 2048 + c0 + n] if m is None
                           else h10[:, m, q * 2048 + c0:q * 2048 + c0 + n]),
                          AF.Tanh, bw2, f"x{g}_{q}")
                for k in range(2):
                    nc.sync.dma_start(
                        hout_d.ap()[k * 128:(k + 1) * 128,
                                    OFF10 + g * 4096:OFF10 + (g + 1) * 4096],
                        h10[:, k, :])
                return h10

            def emit_glevel(g, lv, hchild):
                C = G * (1 << lv)
                if lv == 6:
                    jv6 = jview(6)  # [128, 2, 64*16]
                    out4 = jv6.rearrange("p a (s t) -> p a s t", t=TPC)
                    osl = (lambda c0, n: out4[:, :, :, g * G:(g + 1) * G])
                    hnew = None
                    omax = C  # single chunk
                else:
                    pool = {9: "h9", 8: "h8", 7: "h7"}[lv]
                    hnew = P[pool].tile([128, 2, C], fp16,
                                        name=f"h{lv}g{g}", tag=pool)
                    osl = (lambda c0, n: hnew[:, :, c0:c0 + n])
                    omax = CH_V
                _emit_level(nc, P, f"g{g}l{lv}", G, lv, hchild[:, :, :],
                            osl, omax, Wt, bias, fuse)
                if lv > 6:
                    off = {9: OFF9, 8: OFF8, 7: OFF7}[lv]
                    for k in range(2):
                        nc.sync.dma_start(
                            hout_d.ap()[k * 128:(k + 1) * 128,
                                        off + g * C:off + (g + 1) * C],
                            hnew[:, k, :])
                return hnew

            gstate = {}
            xtiles = {0: load_x(0)}
            for t in range(NG + 5):
                for g in range(NG):
                    s = t - g
                    if s < 0 or s > 4:
                        continue
                    if s == 0:
                        if g + 1 < NG:
                            xtiles[g + 1] = load_x(g + 1)
                        gstate[g] = emit_leaf(g, xtiles.pop(g))
                    else:
                        gstate[g] = emit_glevel(g, 10 - s, gstate[g])

            # level-6 block complete -> stream out
            jv6 = jview(6)
            for k in range(2):
                nc.sync.dma_start(
                    hout_d.ap()[k * 128:(k + 1) * 128,
                                OFFJ + JOFF[6] * TPC:OFFJ + (JOFF[6] + 64) * TPC],
                    jv6[:, k, :])

            # --- joint phase: levels 5..0 over jbuf ---------------------------
            for lv in range(5, -1, -1):
                ov = jview(lv)
                _emit_level(nc, P, f"j{lv}", TPC, lv, jview(lv + 1),
                            (lambda c0, n, ov=ov: ov[:, :, c0:c0 + n]),
                            CH_V, Wt, bias, fuse)
                a = JOFF[lv] * TPC
                for k in range(2):
                    nc.sync.dma_start(
                        hout_d.ap()[k * 128:(k + 1) * 128,
                                    OFFJ + a:OFFJ + a + (1 << lv) * TPC],
                        ov[:, k, :])

    nc.compile()
    return nc


_NC = {}


def _get_nc(fuse=True):
    if fuse not in _NC:
        _NC[fuse] = _build(fuse)
    return _NC[fuse]


def make_in_maps(inputs, fuse):
    x = np.asarray(inputs["x"], np.float32)
    shared = {
        "wT": np.ascontiguousarray(np.asarray(inputs["W"], np.float32).T,
                                   dtype=np.float16),
        "uzT": np.ascontiguousarray(np.asarray(inputs["Uz"], np.float32).T,
                                    dtype=np.float16),
        "urT": np.ascontiguousarray(np.asarray(inputs["Ur"], np.float32).T,
                                    dtype=np.float16),
        "ucT": np.ascontiguousarray(np.asarray(inputs["Uc"], np.float32).T,
                                    dtype=np.float16),
    }
    if not fuse:
        shared.update({
            "bw": np.asarray(inputs["bW"], np.float32).reshape(H, 1),
            "bz": np.asarray(inputs["bz"], np.float32).reshape(H, 1),
            "br": np.asarray(inputs["br"], np.float32).reshape(H, 1),
            "bc": np.asarray(inputs["bc"], np.float32).reshape(H, 1),
        })
    sig10 = SIG[10]
    in_maps = []
    for c in range(NCORES):
        xc = x[c * TPC:(c + 1) * TPC, NLEAF - 1:, :]       # [16, 1024, 256]
        xs = xc[:, sig10, :]                                # slot order
        xT = xs.reshape(NG, G, NLEAF, H).transpose(3, 0, 2, 1).reshape(
            H, TPC * NLEAF)
        in_maps.append({"xT": np.ascontiguousarray(xT, dtype=np.float16),
                        **shared})
    return in_maps


def assemble_out(core_outs):
    out = np.empty((T, NN, H), np.float32)
    for c in range(NCORES):
        ho = np.asarray(core_outs[c])                       # [256, 32752] fp16
        oc = out[c * TPC:(c + 1) * TPC]
        for lv, off in ((10, OFF10), (9, OFF9), (8, OFF8), (7, OFF7)):
            Pl = 1 << lv
            blk = ho[:, off:off + TPC * Pl].reshape(H, NG, Pl, G)
            b = blk.transpose(1, 3, 2, 0).reshape(TPC, Pl, H)
            oc[:, (Pl - 1) + SIG[lv], :] = b.astype(np.float32)
        for lv in range(6, -1, -1):
            Pl = 1 << lv
            a = OFFJ + JOFF[lv] * TPC
            blk = ho[:, a:a + Pl * TPC].reshape(H, Pl, TPC)
            oc[:, (Pl - 1) + SIG[lv], :] = blk.transpose(2, 1, 0).astype(
                np.float32)
    return out


def kernel(**inputs):
    assert int(inputs["depth"]) == DEPTH
    fuse = all(not np.any(np.asarray(inputs[b]))
               for b in ("bW", "br", "bc", "bz"))
    nc = _get_nc(fuse)
    in_maps = make_in_maps(inputs, fuse)
    res = run_bass_kernel_spmd(nc, in_maps, list(range(NCORES)))
    return assemble_out([r["h_out"] for r in res.results])



# revision 3
# speedup vs baseline: 1.1017x; 1.1017x over previous
"""ChildSum TreeGRU on 8 Trainium2 NeuronCores.

Data-parallel over trees (16/core). fp16 on-chip; feature dim folded as
[128 partitions, 2 k-halves, cols]; within each tree level, nodes are stored
in bit-reversed order so children of the (bit-reversed-ordered) parents form
two contiguous blocks [left | right] -> every DVE op is packed/contiguous and
runs in the 2-byte 2x mode.

v2: emission order keeps the PE dense (no slow-engine op ahead of the
r->c matmul chain in any FIFO); h_new = (zh_l - (zsum-1)*c) + zh_r with a
fused scalar_tensor_tensor for the (zsum-1)*c step; 1024-col PSUM blocks
(4 banks) halve the ACT instruction count; the level<=5 "joint" phase runs
as two independent 8-tree chains overlapped with the group-phase wind-down.

Levels 10(leaf)..6 run per group of 4 trees (wavefront across 4 groups);
level-6 results land in per-chain buffers (trees 0-7 / 8-15) holding levels
0..6; levels 5..0 are processed per chain. Host pre-permutes x and
post-permutes the output (bit-reversal + layout), which is free w.r.t. HW
exec time.
"""
import sys

for p in ("/opt/trn_rl_repo", "/root/.axon_site/_ro/trn_rl_repo"):
    if p not in sys.path:
        sys.path.insert(0, p)

import numpy as np
import concourse.tile as tile
from concourse import bacc, mybir
from concourse.bass_utils import run_bass_kernel_spmd

f32 = mybir.dt.float32
fp16 = mybir.dt.float16
AF = mybir.ActivationFunctionType
ALU = mybir.AluOpType

T, DEPTH, NN, H = 128, 11, 2047, 256
NCORES = 8
TPC = T // NCORES          # 16 trees per core
G = 4                      # trees per group
NG = TPC // G              # 4 groups
NLEAF = 1 << (DEPTH - 1)   # 1024

CH_MM = 512                # matmul chunk (one psum bank per (m, chunk))
CH_B = 1024                # psum block cols (4 banks; one ACT per block)
CH_V = 2048                # DVE chunk cols
GPS_MIN = 1024             # split zsum with gpsimd when C >= this

# h_out column blocks (device layout, fp16). Group levels 10..7 then chains.
OFF10 = 0
OFF9 = OFF10 + TPC * 1024
OFF8 = OFF9 + TPC * 512
OFF7 = OFF8 + TPC * 256
OFFJ = OFF7 + TPC * 128            # 30720
JN = 127                            # nodes/tree in chain buf (levels 0..6)
NCH = 2                             # chains (8 trees each)
TPH = TPC // NCH                    # 8 trees per chain
OUTCOLS = OFFJ + NCH * JN * TPH     # 32752
# chain-buf block start (in slots) for level l: levels 6,5,...,0 high->low
JOFF = {l: (JN - ((1 << (l + 1)) - 1)) for l in range(7)}


def _sig(l):
    s = np.zeros(1, np.int64)
    for _ in range(l):
        s = np.concatenate([2 * s, 2 * s + 1])
    return s


SIG = {l: _sig(l) for l in range(DEPTH)}


def _gate(nc, P, wt3, rhs3, cols, out_sl, func, bias2, nm, rh_into=None):
    """out = func(U @ rhs + b) over `cols` columns, in 1024-col psum blocks.

    rhs3: [128, 2, cols] fp16 view. out_sl(c0, n[, m]) -> output AP slice.
    If rh_into is given (r-gate), each block's result is multiplied into
    rh_into (in-place rh = r * hs) right after its activation.
    """
    for c0 in range(0, cols, CH_B):
        n = min(CH_B, cols - c0)
        ps = P["ps"].tile([128, 2, CH_B], f32, name=f"ps{nm}_{c0}", tag="ps")
        for m in range(2):
            for k in range(2):
                for cc in range(0, n, CH_MM):
                    w = min(CH_MM, n - cc)
                    nc.tensor.matmul(ps[:, m, cc:cc + w],
                                     wt3[:, k, m * 128:(m + 1) * 128],
                                     rhs3[:, k, c0 + cc:c0 + cc + w],
                                     start=(k == 0), stop=(k == 1))
        if rh_into is None:
            if bias2 is None:
                nc.scalar.activation(out_sl(c0, n), ps[:, :, :n], func)
            else:
                for m in range(2):
                    nc.scalar.activation(out_sl(c0, n, m), ps[:, m, :n], func,
                                         bias=bias2[m][:])
        else:
            rc = P["rc"].tile([128, 2, CH_B], fp16, name=f"r{nm}_{c0}",
                              tag="rc")
            if bias2 is None:
                nc.scalar.activation(rc[:, :, :n], ps[:, :, :n], func)
            else:
                for m in range(2):
                    nc.scalar.activation(rc[:, m, :n], ps[:, m, :n], func,
                                         bias=bias2[m][:])
            nc.vector.tensor_tensor(rh_into[:, :, c0:c0 + n], rc[:, :, :n],
                                    rh_into[:, :, c0:c0 + n], ALU.mult)


def _emit_level(nc, P, nm, C, lp, hc3, out_sl, out_chunk, Wt, bias, fuse):
    """One GRU level: children hc3 [128, 2, 2C] -> parents via out_sl.

    lp: per-level pools {"z","hs","hc","zs"}. Emission order keeps every
    matmul-feeding op ahead of slow-engine work in each engine FIFO:
      hs(DVE) | z MM/ACT | zsum(GPS+DVE) | r MM/ACT/rh(DVE) | c MM/ACT |
      zh(DVE, in place into z) | t=(zsum-1)*c (STT) | u=zh_l-t | out=u+zh_r
    """
    bz = None if fuse else bias["bz"]
    br = None if fuse else bias["br"]
    bc = None if fuse else bias["bc"]

    # hs = left + right children
    hs = lp["hs"].tile([128, 2, C], fp16, name=f"hs{nm}", tag="t")
    for c0 in range(0, C, CH_V):
        n = min(CH_V, C - c0)
        nc.vector.tensor_tensor(hs[:, :, c0:c0 + n], hc3[:, :, c0:c0 + n],
                                hc3[:, :, C + c0:C + c0 + n], ALU.add)

    # z = sigmoid(Uz @ hc + bz) over all 2C children
    z = lp["z"].tile([128, 2, 2 * C], fp16, name=f"z{nm}", tag="t")
    _gate(nc, P, Wt["uz"], hc3, 2 * C,
          (lambda c0, n, m=None: z[:, :, c0:c0 + n] if m is None
           else z[:, m, c0:c0 + n]),
          AF.Sigmoid, bz, f"z{nm}")

    # zsum = z_l + z_r
    zs = lp["zs"].tile([128, 2, C], fp16, name=f"zs{nm}", tag="t")
    for c0 in range(0, C, CH_V):
        n = min(CH_V, C - c0)
        nc.vector.tensor_tensor(zs[:, :, c0:c0 + n], z[:, :, c0:c0 + n],
                                z[:, :, C + c0:C + c0 + n], ALU.add)

    # r chain: r = sigmoid(Ur @ hs + br); rh = r * hs in place per block
    _gate(nc, P, Wt["ur"], hs[:, :, :], C, None, AF.Sigmoid, br, f"r{nm}",
          rh_into=hs)

    # h_cand = tanh(Uc @ rh + bc)
    hcand = lp["hc"].tile([128, 2, C], fp16, name=f"hc{nm}", tag="t")
    _gate(nc, P, Wt["uc"], hs[:, :, :], C,
          (lambda c0, n, m=None: hcand[:, :, c0:c0 + n] if m is None
           else hcand[:, m, c0:c0 + n]),
          AF.Tanh, bc, f"c{nm}")

    # zh = z * hc in place into z (DVE 2x)
    for c0 in range(0, 2 * C, CH_V):
        n = min(CH_V, 2 * C - c0)
        nc.vector.tensor_tensor(z[:, :, c0:c0 + n], z[:, :, c0:c0 + n],
                                hc3[:, :, c0:c0 + n], ALU.mult)

    # t = (zsum - 1) * hcand, in place into hcand (fused STT)
    for c0 in range(0, C, CH_V):
        n = min(CH_V, C - c0)
        nc.vector.scalar_tensor_tensor(hcand[:, :, c0:c0 + n],
                                       zs[:, :, c0:c0 + n], 1.0,
                                       hcand[:, :, c0:c0 + n],
                                       ALU.subtract, ALU.mult)

    # u = zh_l - t, in place into zh_l; h_new = u + zh_r -> out
    for c0 in range(0, C, CH_V):
        n = min(CH_V, C - c0)
        nc.vector.tensor_tensor(z[:, :, c0:c0 + n], z[:, :, c0:c0 + n],
                                hcand[:, :, c0:c0 + n], ALU.subtract)
    for c0 in range(0, C, out_chunk):
        n = min(out_chunk, C - c0)
        nc.vector.tensor_tensor(out_sl(c0, n), z[:, :, c0:c0 + n],
                                z[:, :, C + c0:C + c0 + n], ALU.add)


def _build(fuse):
    nc = bacc.Bacc("TRN2", debug=False)

    xT_d = nc.dram_tensor("xT", [H, TPC * NLEAF], fp16, kind="ExternalInput")
    wd = {}
    for gnm in ("w", "uz", "ur", "uc"):
        wd[gnm] = nc.dram_tensor(f"{gnm}T", [H, H], fp16, kind="ExternalInput")
    bd = {}
    if not fuse:
        for bnm in ("bw", "bz", "br", "bc"):
            bd[bnm] = nc.dram_tensor(bnm, [H, 1], f32, kind="ExternalInput")
    hout_d = nc.dram_tensor("h_out", [H, OUTCOLS], fp16, kind="ExternalOutput")

    with tile.TileContext(nc) as tc:
        from contextlib import ExitStack
        with ExitStack() as ctx:
            P = {}

            def pool(name, bufs):
                P[name] = ctx.enter_context(tc.tile_pool(name=name, bufs=bufs))

            pool("const", 1)
            pool("xg", 4)
            pool("h10", 2)
            pool("h9", 2)
            pool("h8", 2)
            pool("h7", 2)
            pool("jbA", 1)
            pool("jbB", 1)
            pool("rc", 2)
            # per-level pools (group phase) and per-chain pools (joint phase)
            LP = {}
            for lv in (9, 8, 7, 6):
                for t_ in ("z", "hs", "hc", "zs"):
                    pool(f"{t_}{lv}", 1)
                LP[lv] = {t_: P[f"{t_}{lv}"] for t_ in ("z", "hs", "hc", "zs")}
            for ch in ("A", "B"):
                for t_ in ("z", "hs", "hc", "zs"):
                    pool(f"{t_}J{ch}", 1)
                LP[ch] = {t_: P[f"{t_}J{ch}"]
                          for t_ in ("z", "hs", "hc", "zs")}
            P["ps"] = ctx.enter_context(
                tc.tile_pool(name="ps", bufs=2, space="PSUM"))

            cp = P["const"]
            Wt = {}
            for gnm in ("w", "uz", "ur", "uc"):
                wtile = cp.tile([128, 2, H], fp16, name=f"{gnm}T")
                Wt[gnm] = wtile
                for k in range(2):
                    nc.sync.dma_start(wtile[:, k, :],
                                      wd[gnm].ap()[k * 128:(k + 1) * 128, :])
            bias = {}
            if not fuse:
                for bnm in ("bw", "bz", "br", "bc"):
                    bias[bnm] = [cp.tile([128, 1], f32, name=f"{bnm}{m}")
                                 for m in range(2)]
                    for m in range(2):
                        nc.sync.dma_start(bias[bnm][m][:],
                                          bd[bnm].ap()[m * 128:(m + 1) * 128, :])
            bw2 = None if fuse else bias["bw"]

            jb = {"A": P["jbA"].tile([128, 2, JN * TPH], fp16, name="jbA"),
                  "B": P["jbB"].tile([128, 2, JN * TPH], fp16, name="jbB")}

            def jview(ch, l):
                a = JOFF[l] * TPH
                return jb[ch][:, :, a:a + (1 << l) * TPH]

            # --- group phase -------------------------------------------------
            def load_x(g):
                ts = []
                for q in range(2):
                    xt = P["xg"].tile([128, 2, 2048], fp16,
                                      name=f"x{g}_{q}", tag="xg")
                    for k in range(2):
                        c0 = g * G * NLEAF + q * 2048
                        nc.sync.dma_start(xt[:, k, :],
                                          xT_d.ap()[k * 128:(k + 1) * 128,
                                                    c0:c0 + 2048])
                    ts.append(xt)
                return ts

            def emit_leaf(g, xts):
                h10 = P["h10"].tile([128, 2, G * NLEAF], fp16,
                                    name=f"h10g{g}", tag="h10")
                for q in range(2):
                    _gate(nc, P, Wt["w"], xts[q][:, :, :], 2048,
                          (lambda c0, n, m=None, q=q:
                           h10[:, :, q * 2048 + c0:q * 2048 + c0 + n] if m is None
                           else h10[:, m, q * 2048 + c0:q * 2048 + c0 + n]),
                          AF.Tanh, bw2, f"x{g}_{q}")
                for k in range(2):
                    nc.sync.dma_start(
                        hout_d.ap()[k * 128:(k + 1) * 128,
                                    OFF10 + g * 4096:OFF10 + (g + 1) * 4096],
                        h10[:, k, :])
                return h10

            def emit_glevel(g, lv, hchild):
                C = G * (1 << lv)
                if lv == 6:
                    ch = "A" if g < 2 else "B"
                    off = (g % 2) * G
                    jv6 = jview(ch, 6)  # [128, 2, 64*8]
                    out4 = jv6.rearrange("p a (s t) -> p a s t", t=TPH)
                    osl = (lambda c0, n: out4[:, :, :, off:off + G])
                    hnew = None
                    ochunk = C  # single chunk
                else:
                    pool_ = {9: "h9", 8: "h8", 7: "h7"}[lv]
                    hnew = P[pool_].tile([128, 2, C], fp16,
                                         name=f"h{lv}g{g}", tag=pool_)
                    osl = (lambda c0, n: hnew[:, :, c0:c0 + n])
                    ochunk = CH_B
                _emit_level(nc, P, f"g{g}l{lv}", C, LP[lv], hchild[:, :, :],
                            osl, ochunk, Wt, bias, fuse)
                if lv > 6:
                    off = {9: OFF9, 8: OFF8, 7: OFF7}[lv]
                    for k in range(2):
                        nc.sync.dma_start(
                            hout_d.ap()[k * 128:(k + 1) * 128,
                                        off + g * C:off + (g + 1) * C],
                            hnew[:, k, :])
                return hnew

            def emit_jlevel(ch, lv):
                C = TPH * (1 << lv)
                ov = jview(ch, lv)
                _emit_level(nc, P, f"j{ch}{lv}", C, LP[ch],
                            jview(ch, lv + 1),
                            (lambda c0, n, ov=ov: ov[:, :, c0:c0 + n]),
                            CH_B, Wt, bias, fuse)

            # wavefront: group g runs step s = t - g (leaf, lv9..lv6);
            # chain A (groups 0,1) starts after tick 5, B after tick 7.
            gstate = {}
            xtiles = {0: load_x(0)}
            for t in range(14):
                for g in range(NG):
                    s = t - g
                    if s < 0 or s > 4:
                        continue
                    if s == 0:
                        if g + 1 < NG:
                            xtiles[g + 1] = load_x(g + 1)
                        gstate[g] = emit_leaf(g, xtiles.pop(g))
                    else:
                        gstate[g] = emit_glevel(g, 10 - s, gstate[g])
                if 6 <= t <= 11:
                    emit_jlevel("A", 11 - t)
                if 8 <= t <= 13:
                    emit_jlevel("B", 13 - t)

            # stream chain buffers out
            for ci, ch in enumerate(("A", "B")):
                a = OFFJ + ci * JN * TPH
                for k in range(2):
                    nc.sync.dma_start(
                        hout_d.ap()[k * 128:(k + 1) * 128, a:a + JN * TPH],
                        jb[ch][:, k, :])

    nc.compile()
    return nc


_NC = {}


def _get_nc(fuse=True):
    if fuse not in _NC:
        _NC[fuse] = _build(fuse)
    return _NC[fuse]


def make_in_maps(inputs, fuse):
    x = np.asarray(inputs["x"], np.float32)
    shared = {
        "wT": np.ascontiguousarray(np.asarray(inputs["W"], np.float32).T,
                                   dtype=np.float16),
        "uzT": np.ascontiguousarray(np.asarray(inputs["Uz"], np.float32).T,
                                    dtype=np.float16),
        "urT": np.ascontiguousarray(np.asarray(inputs["Ur"], np.float32).T,
                                    dtype=np.float16),
        "ucT": np.ascontiguousarray(np.asarray(inputs["Uc"], np.float32).T,
                                    dtype=np.float16),
    }
    if not fuse:
        shared.update({
            "bw": np.asarray(inputs["bW"], np.float32).reshape(H, 1),
            "bz": np.asarray(inputs["bz"], np.float32).reshape(H, 1),
            "br": np.asarray(inputs["br"], np.float32).reshape(H, 1),
            "bc": np.asarray(inputs["bc"], np.float32).reshape(H, 1),
        })
    sig10 = SIG[10]
    in_maps = []
    for c in range(NCORES):
        xc = x[c * TPC:(c + 1) * TPC, NLEAF - 1:, :]       # [16, 1024, 256]
        xs = xc[:, sig10, :]                                # slot order
        xT = xs.reshape(NG, G, NLEAF, H).transpose(3, 0, 2, 1).reshape(
            H, TPC * NLEAF)
        in_maps.append({"xT": np.ascontiguousarray(xT, dtype=np.float16),
                        **shared})
    return in_maps


def assemble_out(core_outs):
    out = np.empty((T, NN, H), np.float32)
    for c in range(NCORES):
        ho = np.asarray(core_outs[c])                       # [256, 32752] fp16
        oc = out[c * TPC:(c + 1) * TPC]
        for lv, off in ((10, OFF10), (9, OFF9), (8, OFF8), (7, OFF7)):
            Pl = 1 << lv
            blk = ho[:, off:off + TPC * Pl].reshape(H, NG, Pl, G)
            b = blk.transpose(1, 3, 2, 0).reshape(TPC, Pl, H)
            oc[:, (Pl - 1) + SIG[lv], :] = b.astype(np.float32)
        for ci in range(NCH):
            tc0 = ci * TPH
            for lv in range(6, -1, -1):
                Pl = 1 << lv
                a = OFFJ + ci * JN * TPH + JOFF[lv] * TPH
                blk = ho[:, a:a + Pl * TPH].reshape(H, Pl, TPH)
                oc[tc0:tc0 + TPH, (Pl - 1) + SIG[lv], :] = blk.transpose(
                    2, 1, 0).astype(np.float32)
    return out


def kernel(**inputs):
    assert int(inputs["depth"]) == DEPTH
    fuse = all(not np.any(np.asarray(inputs[b]))
               for b in ("bW", "br", "bc", "bz"))
    nc = _get_nc(fuse)
    in_maps = make_in_maps(inputs, fuse)
    res = run_bass_kernel_spmd(nc, in_maps, list(range(NCORES)))
    return assemble_out([r["h_out"] for r in res.results])


# revision 4
# speedup vs baseline: 1.2585x; 1.1423x over previous
"""ChildSum TreeGRU on 8 Trainium2 NeuronCores.

Data-parallel over trees (16/core). fp16 on-chip; feature dim folded as
[128 partitions, 2 k-halves, cols]; within each tree level, nodes are stored
in bit-reversed order so children of the (bit-reversed-ordered) parents form
two contiguous blocks [left | right] -> every DVE op is packed/contiguous and
runs in the 2-byte 2x mode.

v3: split-phase emission — each tick emits the front half (hs, z-gate,
r-gate+rh) of every active step before any back half (c-gate + tail), so
other steps' matmuls fill the ACT->rh latency window ahead of each c-gate
and the PE never idles long enough for the HAM clock gate to re-throttle.
h_new = (zh_l - (zsum-1)*c) + zh_r with a fused scalar_tensor_tensor.
Levels with 3C <= 1024 put z|r in one PSUM tile and run a single merged
sigmoid ACT (one less chain hop). 1024-col PSUM blocks (4 banks) halve the
ACT instruction count. The level<=5 phase runs as two independent 8-tree
chains overlapped with the group-phase wind-down.

Levels 10(leaf)..6 run per group of 4 trees (wavefront across 4 groups).
Host pre-permutes x and post-permutes the output (bit-reversal + layout),
which is free w.r.t. HW exec time.
"""
import sys

for p in ("/opt/trn_rl_repo", "/root/.axon_site/_ro/trn_rl_repo"):
    if p not in sys.path:
        sys.path.insert(0, p)

import numpy as np
import concourse.tile as tile
from concourse import bacc, mybir
from concourse.bass_utils import run_bass_kernel_spmd

f32 = mybir.dt.float32
fp16 = mybir.dt.float16
AF = mybir.ActivationFunctionType
ALU = mybir.AluOpType

T, DEPTH, NN, H = 128, 11, 2047, 256
NCORES = 8
TPC = T // NCORES          # 16 trees per core
G = 4                      # trees per group
NG = TPC // G              # 4 groups
NLEAF = 1 << (DEPTH - 1)   # 1024

CH_MM = 512                # matmul chunk (one psum bank per (m, chunk))
CH_B = 1024                # psum block cols (4 banks; one ACT per block)
CH_V = 2048                # DVE chunk cols

# h_out column blocks (device layout, fp16). Group levels 10..7 then chains.
OFF10 = 0
OFF9 = OFF10 + TPC * 1024
OFF8 = OFF9 + TPC * 512
OFF7 = OFF8 + TPC * 256
OFFJ = OFF7 + TPC * 128            # 30720
JN = 127                            # nodes/tree in chain buf (levels 0..6)
NCH = 2                             # chains (8 trees each)
TPH = TPC // NCH                    # 8 trees per chain
OUTCOLS = OFFJ + NCH * JN * TPH     # 32752
# chain-buf block start (in slots) for level l: levels 6,5,...,0 high->low
JOFF = {l: (JN - ((1 << (l + 1)) - 1)) for l in range(7)}


def _sig(l):
    s = np.zeros(1, np.int64)
    for _ in range(l):
        s = np.concatenate([2 * s, 2 * s + 1])
    return s


SIG = {l: _sig(l) for l in range(DEPTH)}


def _gate(nc, P, wt3, rhs3, cols, out_sl, func, bias2, nm, rh_into=None):
    """out = func(U @ rhs + b) over `cols` columns, in 1024-col psum blocks."""
    for c0 in range(0, cols, CH_B):
        n = min(CH_B, cols - c0)
        ps = P["ps"].tile([128, 2, CH_B], f32, name=f"ps{nm}_{c0}", tag="ps")
        for m in range(2):
            for k in range(2):
                for cc in range(0, n, CH_MM):
                    w = min(CH_MM, n - cc)
                    nc.tensor.matmul(ps[:, m, cc:cc + w],
                                     wt3[:, k, m * 128:(m + 1) * 128],
                                     rhs3[:, k, c0 + cc:c0 + cc + w],
                                     start=(k == 0), stop=(k == 1))
        if rh_into is None:
            if bias2 is None:
                nc.scalar.activation(out_sl(c0, n), ps[:, :, :n], func)
            else:
                for m in range(2):
                    nc.scalar.activation(out_sl(c0, n, m), ps[:, m, :n], func,
                                         bias=bias2[m][:])
        else:
            rc = P["rc"].tile([128, 2, CH_B], fp16, name=f"r{nm}_{c0}",
                              tag="rc")
            if bias2 is None:
                nc.scalar.activation(rc[:, :, :n], ps[:, :, :n], func)
            else:
                for m in range(2):
                    nc.scalar.activation(rc[:, m, :n], ps[:, m, :n], func,
                                         bias=bias2[m][:])
            nc.vector.tensor_tensor(rh_into[:, :, c0:c0 + n], rc[:, :, :n],
                                    rh_into[:, :, c0:c0 + n], ALU.mult)


def _mm_cols(nc, ps, wt3, rhs3, cols, p0):
    """Matmuls of wt3 @ rhs3 (all `cols`) into psum block at col offset p0."""
    for m in range(2):
        for k in range(2):
            for cc in range(0, cols, CH_MM):
                w = min(CH_MM, cols - cc)
                nc.tensor.matmul(ps[:, m, p0 + cc:p0 + cc + w],
                                 wt3[:, k, m * 128:(m + 1) * 128],
                                 rhs3[:, k, cc:cc + w],
                                 start=(k == 0), stop=(k == 1))


def _level_front(nc, P, nm, C, lp, hc3, Wt, bias, fuse):
    """hs, z-gate, r-gate (+rh into hs), zsum. Returns state for the back."""
    bz = None if fuse else bias["bz"]
    br = None if fuse else bias["br"]

    hs = lp["hs"].tile([128, 2, C], fp16, name=f"hs{nm}", tag="t")
    for c0 in range(0, C, CH_V):
        n = min(CH_V, C - c0)
        nc.vector.tensor_tensor(hs[:, :, c0:c0 + n], hc3[:, :, c0:c0 + n],
                                hc3[:, :, C + c0:C + c0 + n], ALU.add)

    if 3 * C <= CH_B and fuse:
        # merged z|r sigmoid: one psum block [z(2C) | r(C)], one ACT
        zr = lp["z"].tile([128, 2, 3 * C], fp16, name=f"zr{nm}", tag="t")
        ps = P["ps"].tile([128, 2, CH_B], f32, name=f"pzr{nm}", tag="ps")
        _mm_cols(nc, ps, Wt["uz"], hc3, 2 * C, 0)
        _mm_cols(nc, ps, Wt["ur"], hs, C, 2 * C)
        nc.scalar.activation(zr[:, :, :], ps[:, :, :3 * C], AF.Sigmoid)
        z = zr[:, :, :2 * C]
        nc.vector.tensor_tensor(hs[:, :, :], zr[:, :, 2 * C:3 * C],
                                hs[:, :, :], ALU.mult)
    else:
        zt = lp["z"].tile([128, 2, 2 * C], fp16, name=f"z{nm}", tag="t")
        z = zt[:, :, :]
        _gate(nc, P, Wt["uz"], hc3, 2 * C,
              (lambda c0, n, m=None: zt[:, :, c0:c0 + n] if m is None
               else zt[:, m, c0:c0 + n]),
              AF.Sigmoid, bz, f"z{nm}")
        _gate(nc, P, Wt["ur"], hs[:, :, :], C, None, AF.Sigmoid, br, f"r{nm}",
              rh_into=hs)

    zs = lp["zs"].tile([128, 2, C], fp16, name=f"zs{nm}", tag="t")
    for c0 in range(0, C, CH_V):
        n = min(CH_V, C - c0)
        nc.vector.tensor_tensor(zs[:, :, c0:c0 + n], z[:, :, c0:c0 + n],
                                z[:, :, C + c0:C + c0 + n], ALU.add)

    return {"hs": hs, "z": z, "zs": zs, "hc3": hc3, "C": C, "nm": nm,
            "lp": lp}


def _level_back(nc, P, st, out_sl, out_chunk, Wt, bias, fuse):
    """c-gate, zh, t=(zsum-1)*c, u=zh_l-t, h_new=u+zh_r -> out."""
    C, nm, lp = st["C"], st["nm"], st["lp"]
    hs, z, zs, hc3 = st["hs"], st["z"], st["zs"], st["hc3"]
    bc = None if fuse else bias["bc"]

    hcand = lp["hc"].tile([128, 2, C], fp16, name=f"hc{nm}", tag="t")
    _gate(nc, P, Wt["uc"], hs[:, :, :], C,
          (lambda c0, n, m=None: hcand[:, :, c0:c0 + n] if m is None
           else hcand[:, m, c0:c0 + n]),
          AF.Tanh, bc, f"c{nm}")

    # zh = z * hc in place into z (DVE 2x)
    for c0 in range(0, 2 * C, CH_V):
        n = min(CH_V, 2 * C - c0)
        nc.vector.tensor_tensor(z[:, :, c0:c0 + n], z[:, :, c0:c0 + n],
                                hc3[:, :, c0:c0 + n], ALU.mult)

    # t = (zsum - 1) * hcand, in place into hcand (fused STT)
    for c0 in range(0, C, CH_V):
        n = min(CH_V, C - c0)
        nc.vector.scalar_tensor_tensor(hcand[:, :, c0:c0 + n],
                                       zs[:, :, c0:c0 + n], 1.0,
                                       hcand[:, :, c0:c0 + n],
                                       ALU.subtract, ALU.mult)

    # u = zh_l - t, in place into zh_l; h_new = u + zh_r -> out
    for c0 in range(0, C, CH_V):
        n = min(CH_V, C - c0)
        nc.vector.tensor_tensor(z[:, :, c0:c0 + n], z[:, :, c0:c0 + n],
                                hcand[:, :, c0:c0 + n], ALU.subtract)
    for c0 in range(0, C, out_chunk):
        n = min(out_chunk, C - c0)
        nc.vector.tensor_tensor(out_sl(c0, n), z[:, :, c0:c0 + n],
                                z[:, :, C + c0:C + c0 + n], ALU.add)


def _build(fuse):
    nc = bacc.Bacc("TRN2", debug=False)

    xT_d = nc.dram_tensor("xT", [H, TPC * NLEAF], fp16, kind="ExternalInput")
    wd = {}
    for gnm in ("w", "uz", "ur", "uc"):
        wd[gnm] = nc.dram_tensor(f"{gnm}T", [H, H], fp16, kind="ExternalInput")
    bd = {}
    if not fuse:
        for bnm in ("bw", "bz", "br", "bc"):
            bd[bnm] = nc.dram_tensor(bnm, [H, 1], f32, kind="ExternalInput")
    hout_d = nc.dram_tensor("h_out", [H, OUTCOLS], fp16, kind="ExternalOutput")

    with tile.TileContext(nc) as tc:
        from contextlib import ExitStack
        with ExitStack() as ctx:
            P = {}

            def pool(name, bufs):
                P[name] = ctx.enter_context(tc.tile_pool(name=name, bufs=bufs))

            pool("const", 1)
            pool("xg", 4)
            pool("h10", 2)
            pool("h9", 2)
            pool("h8", 2)
            pool("h7", 2)
            pool("jbA", 1)
            pool("jbB", 1)
            pool("rc", 2)
            LP = {}
            for lv in (9, 8, 7, 6):
                for t_ in ("z", "hs", "hc", "zs"):
                    pool(f"{t_}{lv}", 1)
                LP[lv] = {t_: P[f"{t_}{lv}"] for t_ in ("z", "hs", "hc", "zs")}
            for ch in ("A", "B"):
                for t_ in ("z", "hs", "hc", "zs"):
                    pool(f"{t_}J{ch}", 1)
                LP[ch] = {t_: P[f"{t_}J{ch}"]
                          for t_ in ("z", "hs", "hc", "zs")}
            P["ps"] = ctx.enter_context(
                tc.tile_pool(name="ps", bufs=2, space="PSUM"))

            cp = P["const"]
            Wt = {}
            for gnm in ("w", "uz", "ur", "uc"):
                wtile = cp.tile([128, 2, H], fp16, name=f"{gnm}T")
                Wt[gnm] = wtile
                for k in range(2):
                    nc.sync.dma_start(wtile[:, k, :],
                                      wd[gnm].ap()[k * 128:(k + 1) * 128, :])
            bias = {}
            if not fuse:
                for bnm in ("bw", "bz", "br", "bc"):
                    bias[bnm] = [cp.tile([128, 1], f32, name=f"{bnm}{m}")
                                 for m in range(2)]
                    for m in range(2):
                        nc.sync.dma_start(bias[bnm][m][:],
                                          bd[bnm].ap()[m * 128:(m + 1) * 128, :])
            bw2 = None if fuse else bias["bw"]

            jb = {"A": P["jbA"].tile([128, 2, JN * TPH], fp16, name="jbA"),
                  "B": P["jbB"].tile([128, 2, JN * TPH], fp16, name="jbB")}

            def jview(ch, l):
                a = JOFF[l] * TPH
                return jb[ch][:, :, a:a + (1 << l) * TPH]

            def load_x(g):
                ts = []
                for q in range(2):
                    xt = P["xg"].tile([128, 2, 2048], fp16,
                                      name=f"x{g}_{q}", tag="xg")
                    for k in range(2):
                        c0 = g * G * NLEAF + q * 2048
                        nc.sync.dma_start(xt[:, k, :],
                                          xT_d.ap()[k * 128:(k + 1) * 128,
                                                    c0:c0 + 2048])
                    ts.append(xt)
                return ts

            def emit_leaf(g, xts):
                h10 = P["h10"].tile([128, 2, G * NLEAF], fp16,
                                    name=f"h10g{g}", tag="h10")
                for q in range(2):
                    _gate(nc, P, Wt["w"], xts[q][:, :, :], 2048,
                          (lambda c0, n, m=None, q=q:
                           h10[:, :, q * 2048 + c0:q * 2048 + c0 + n] if m is None
                           else h10[:, m, q * 2048 + c0:q * 2048 + c0 + n]),
                          AF.Tanh, bw2, f"x{g}_{q}")
                for k in range(2):
                    nc.sync.dma_start(
                        hout_d.ap()[k * 128:(k + 1) * 128,
                                    OFF10 + g * 4096:OFF10 + (g + 1) * 4096],
                        h10[:, k, :])
                return h10

            def back_glevel(g, lv, st):
                C = st["C"]
                if lv == 6:
                    ch = "A" if g < 2 else "B"
                    off = (g % 2) * G
                    jv6 = jview(ch, 6)
                    out4 = jv6.rearrange("p a (s t) -> p a s t", t=TPH)
                    osl = (lambda c0, n: out4[:, :, :, off:off + G])
                    _level_back(nc, P, st, osl, C, Wt, bias, fuse)
                    return None
                pool_ = {9: "h9", 8: "h8", 7: "h7"}[lv]
                hnew = P[pool_].tile([128, 2, C], fp16,
                                     name=f"h{lv}g{g}", tag=pool_)
                _level_back(nc, P, st,
                            (lambda c0, n: hnew[:, :, c0:c0 + n]),
                            CH_B, Wt, bias, fuse)
                off = {9: OFF9, 8: OFF8, 7: OFF7}[lv]
                for k in range(2):
                    nc.sync.dma_start(
                        hout_d.ap()[k * 128:(k + 1) * 128,
                                    off + g * C:off + (g + 1) * C],
                        hnew[:, k, :])
                return hnew

            # wavefront with split-phase emission per tick
            gstate = {}
            xtiles = {0: load_x(0)}
            for t in range(14):
                items = []
                for g in range(NG):
                    s = t - g
                    if 1 <= s <= 4:
                        items.append(("g", g, 10 - s))
                if 6 <= t <= 11:
                    items.append(("c", "A", 11 - t))
                if 8 <= t <= 13:
                    items.append(("c", "B", 13 - t))

                sts = []
                for kind, gg, lv in items:
                    if kind == "g":
                        C = G * (1 << lv)
                        st = _level_front(nc, P, f"g{gg}l{lv}", C, LP[lv],
                                          gstate[gg][:, :, :], Wt, bias, fuse)
                    else:
                        C = TPH * (1 << lv)
                        st = _level_front(nc, P, f"j{gg}{lv}", C, LP[gg],
                                          jview(gg, lv + 1), Wt, bias, fuse)
                    sts.append(st)

                for (kind, gg, lv), st in zip(items, sts):
                    if kind == "g":
                        gstate[gg] = back_glevel(gg, lv, st)
                    else:
                        ov = jview(gg, lv)
                        _level_back(nc, P, st,
                                    (lambda c0, n, ov=ov: ov[:, :, c0:c0 + n]),
                                    CH_B, Wt, bias, fuse)

                for g in range(NG):
                    if t - g == 0:
                        if g + 1 < NG:
                            xtiles[g + 1] = load_x(g + 1)
                        gstate[g] = emit_leaf(g, xtiles.pop(g))

            # stream chain buffers out
            for ci, ch in enumerate(("A", "B")):
                a = OFFJ + ci * JN * TPH
                for k in range(2):
                    nc.sync.dma_start(
                        hout_d.ap()[k * 128:(k + 1) * 128, a:a + JN * TPH],
                        jb[ch][:, k, :])

    nc.compile()
    return nc


_NC = {}


def _get_nc(fuse=True):
    if fuse not in _NC:
        _NC[fuse] = _build(fuse)
    return _NC[fuse]


def make_in_maps(inputs, fuse):
    x = np.asarray(inputs["x"], np.float32)
    shared = {
        "wT": np.ascontiguousarray(np.asarray(inputs["W"], np.float32).T,
                                   dtype=np.float16),
        "uzT": np.ascontiguousarray(np.asarray(inputs["Uz"], np.float32).T,
                                    dtype=np.float16),
        "urT": np.ascontiguousarray(np.asarray(inputs["Ur"], np.float32).T,
                                    dtype=np.float16),
        "ucT": np.ascontiguousarray(np.asarray(inputs["Uc"], np.float32).T,
                                    dtype=np.float16),
    }
    if not fuse:
        shared.update({
            "bw": np.asarray(inputs["bW"], np.float32).reshape(H, 1),
            "bz": np.asarray(inputs["bz"], np.float32).reshape(H, 1),
            "br": np.asarray(inputs["br"], np.float32).reshape(H, 1),
            "bc": np.asarray(inputs["bc"], np.float32).reshape(H, 1),
        })
    sig10 = SIG[10]
    in_maps = []
    for c in range(NCORES):
        xc = x[c * TPC:(c + 1) * TPC, NLEAF - 1:, :]       # [16, 1024, 256]
        xs = xc[:, sig10, :]                                # slot order
        xT = xs.reshape(NG, G, NLEAF, H).transpose(3, 0, 2, 1).reshape(
            H, TPC * NLEAF)
        in_maps.append({"xT": np.ascontiguousarray(xT, dtype=np.float16),
                        **shared})
    return in_maps


def assemble_out(core_outs):
    out = np.empty((T, NN, H), np.float32)
    for c in range(NCORES):
        ho = np.asarray(core_outs[c])                       # [256, 32752] fp16
        oc = out[c * TPC:(c + 1) * TPC]
        for lv, off in ((10, OFF10), (9, OFF9), (8, OFF8), (7, OFF7)):
            Pl = 1 << lv
            blk = ho[:, off:off + TPC * Pl].reshape(H, NG, Pl, G)
            b = blk.transpose(1, 3, 2, 0).reshape(TPC, Pl, H)
            oc[:, (Pl - 1) + SIG[lv], :] = b.astype(np.float32)
        for ci in range(NCH):
            tc0 = ci * TPH
            for lv in range(6, -1, -1):
                Pl = 1 << lv
                a = OFFJ + ci * JN * TPH + JOFF[lv] * TPH
                blk = ho[:, a:a + Pl * TPH].reshape(H, Pl, TPH)
                oc[tc0:tc0 + TPH, (Pl - 1) + SIG[lv], :] = blk.transpose(
                    2, 1, 0).astype(np.float32)
    return out


def kernel(**inputs):
    assert int(inputs["depth"]) == DEPTH
    fuse = all(not np.any(np.asarray(inputs[b]))
               for b in ("bW", "br", "bc", "bz"))
    nc = _get_nc(fuse)
    in_maps = make_in_maps(inputs, fuse)
    res = run_bass_kernel_spmd(nc, in_maps, list(range(NCORES)))
    return assemble_out([r["h_out"] for r in res.results])


# revision 8
# speedup vs baseline: 1.2637x; 1.0041x over previous
"""ChildSum TreeGRU on 8 Trainium2 NeuronCores.

Data-parallel over trees (16/core). fp16 on-chip; feature dim folded as
[128 partitions, 2 k-halves, cols]; within each tree level, nodes are stored
in bit-reversed order so children of the (bit-reversed-ordered) parents form
two contiguous blocks [left | right] -> every DVE op is packed/contiguous and
runs in the 2-byte 2x mode.

v3: split-phase emission — each tick emits the front half (hs, z-gate,
r-gate+rh) of every active step before any back half (c-gate + tail), so
other steps' matmuls fill the ACT->rh latency window ahead of each c-gate
and the PE never idles long enough for the HAM clock gate to re-throttle.
h_new = (zh_l - (zsum-1)*c) + zh_r with a fused scalar_tensor_tensor.
Levels with 3C <= 1024 put z|r in one PSUM tile and run a single merged
sigmoid ACT (one less chain hop). 1024-col PSUM blocks (4 banks) halve the
ACT instruction count. The level<=5 phase runs as two independent 8-tree
chains overlapped with the group-phase wind-down.

Levels 10(leaf)..6 run per group of 4 trees (wavefront across 4 groups).
Host pre-permutes x and post-permutes the output (bit-reversal + layout),
which is free w.r.t. HW exec time.
"""
import sys

for p in ("/opt/trn_rl_repo", "/root/.axon_site/_ro/trn_rl_repo"):
    if p not in sys.path:
        sys.path.insert(0, p)

import numpy as np
import concourse.tile as tile
from concourse import bacc, mybir
from concourse.bass_utils import run_bass_kernel_spmd

f32 = mybir.dt.float32
fp16 = mybir.dt.float16
AF = mybir.ActivationFunctionType
ALU = mybir.AluOpType

T, DEPTH, NN, H = 128, 11, 2047, 256
NCORES = 8
TPC = T // NCORES          # 16 trees per core
G = 4                      # trees per group
NG = TPC // G              # 4 groups
NLEAF = 1 << (DEPTH - 1)   # 1024

CH_MM = 512                # matmul chunk (one psum bank per (m, chunk))
CH_B = 1024                # psum block cols (4 banks; one ACT per block)
CH_V = 2048                # DVE chunk cols

# h_out column blocks (device layout, fp16). Group levels 10..7 then chains.
OFF10 = 0
OFF9 = OFF10 + TPC * 1024
OFF8 = OFF9 + TPC * 512
OFF7 = OFF8 + TPC * 256
OFFJ = OFF7 + TPC * 128            # 30720
JN = 127                            # nodes/tree in chain buf (levels 0..6)
NCH = 2                             # chains (8 trees each)
TPH = TPC // NCH                    # 8 trees per chain
OUTCOLS = OFFJ + NCH * JN * TPH     # 32752
# chain-buf block start (in slots) for level l: levels 6,5,...,0 high->low
JOFF = {l: (JN - ((1 << (l + 1)) - 1)) for l in range(7)}


def _sig(l):
    s = np.zeros(1, np.int64)
    for _ in range(l):
        s = np.concatenate([2 * s, 2 * s + 1])
    return s


SIG = {l: _sig(l) for l in range(DEPTH)}


def _gate(nc, P, wt3, rhs3, cols, out_sl, func, bias2, nm, rh_into=None):
    """out = func(U @ rhs + b) over `cols` columns, in 1024-col psum blocks."""
    for c0 in range(0, cols, CH_B):
        n = min(CH_B, cols - c0)
        ps = P["ps"].tile([128, 2, CH_B], f32, name=f"ps{nm}_{c0}", tag="ps")
        for m in range(2):
            for k in range(2):
                for cc in range(0, n, CH_MM):
                    w = min(CH_MM, n - cc)
                    nc.tensor.matmul(ps[:, m, cc:cc + w],
                                     wt3[:, k, m * 128:(m + 1) * 128],
                                     rhs3[:, k, c0 + cc:c0 + cc + w],
                                     start=(k == 0), stop=(k == 1))
        if rh_into is None:
            if bias2 is None:
                nc.scalar.activation(out_sl(c0, n), ps[:, :, :n], func)
            else:
                for m in range(2):
                    nc.scalar.activation(out_sl(c0, n, m), ps[:, m, :n], func,
                                         bias=bias2[m][:])
        else:
            rc = P["rc"].tile([128, 2, CH_B], fp16, name=f"r{nm}_{c0}",
                              tag="rc")
            if bias2 is None:
                nc.scalar.activation(rc[:, :, :n], ps[:, :, :n], func)
            else:
                for m in range(2):
                    nc.scalar.activation(rc[:, m, :n], ps[:, m, :n], func,
                                         bias=bias2[m][:])
            nc.vector.tensor_tensor(rh_into[:, :, c0:c0 + n], rc[:, :, :n],
                                    rh_into[:, :, c0:c0 + n], ALU.mult)


def _mm_cols(nc, ps, wt3, rhs3, cols, p0):
    """Matmuls of wt3 @ rhs3 (all `cols`) into psum block at col offset p0."""
    for m in range(2):
        for k in range(2):
            for cc in range(0, cols, CH_MM):
                w = min(CH_MM, cols - cc)
                nc.tensor.matmul(ps[:, m, p0 + cc:p0 + cc + w],
                                 wt3[:, k, m * 128:(m + 1) * 128],
                                 rhs3[:, k, cc:cc + w],
                                 start=(k == 0), stop=(k == 1))


def _level_front(nc, P, nm, C, lp, hc3, Wt, bias, fuse):
    """hs, z-gate, r-gate (+rh into hs), zsum. Returns state for the back."""
    bz = None if fuse else bias["bz"]
    br = None if fuse else bias["br"]

    hs = lp["hs"].tile([128, 2, C], fp16, name=f"hs{nm}", tag="t")
    for c0 in range(0, C, CH_V):
        n = min(CH_V, C - c0)
        nc.vector.tensor_tensor(hs[:, :, c0:c0 + n], hc3[:, :, c0:c0 + n],
                                hc3[:, :, C + c0:C + c0 + n], ALU.add)

    if 3 * C <= CH_B and fuse:
        # merged z|r sigmoid: one psum block [z(2C) | r(C)], one ACT
        zr = lp["z"].tile([128, 2, 3 * C], fp16, name=f"zr{nm}", tag="t")
        ps = P["ps"].tile([128, 2, CH_B], f32, name=f"pzr{nm}", tag="ps")
        _mm_cols(nc, ps, Wt["uz"], hc3, 2 * C, 0)
        _mm_cols(nc, ps, Wt["ur"], hs, C, 2 * C)
        nc.scalar.activation(zr[:, :, :], ps[:, :, :3 * C], AF.Sigmoid)
        z = zr[:, :, :2 * C]
        nc.vector.tensor_tensor(hs[:, :, :], zr[:, :, 2 * C:3 * C],
                                hs[:, :, :], ALU.mult)
    else:
        zt = lp["z"].tile([128, 2, 2 * C], fp16, name=f"z{nm}", tag="t")
        z = zt[:, :, :]
        _gate(nc, P, Wt["uz"], hc3, 2 * C,
              (lambda c0, n, m=None: zt[:, :, c0:c0 + n] if m is None
               else zt[:, m, c0:c0 + n]),
              AF.Sigmoid, bz, f"z{nm}")
        _gate(nc, P, Wt["ur"], hs[:, :, :], C, None, AF.Sigmoid, br, f"r{nm}",
              rh_into=hs)

    zs = lp["zs"].tile([128, 2, C], fp16, name=f"zs{nm}", tag="t")
    c0 = 0
    if C >= 1024:
        # gpsimd takes the first half in 512-col chunks (proven size)
        for c0 in range(0, C // 2, 512):
            nc.gpsimd.tensor_tensor(zs[:, :, c0:c0 + 512],
                                    z[:, :, c0:c0 + 512],
                                    z[:, :, C + c0:C + c0 + 512], ALU.add)
        c0 = C // 2
    for c0 in range(c0, C, CH_V):
        n = min(CH_V, C - c0)
        nc.vector.tensor_tensor(zs[:, :, c0:c0 + n], z[:, :, c0:c0 + n],
                                z[:, :, C + c0:C + c0 + n], ALU.add)

    return {"hs": hs, "z": z, "zs": zs, "hc3": hc3, "C": C, "nm": nm,
            "lp": lp}


def _level_back(nc, P, st, out_sl, out_chunk, Wt, bias, fuse):
    """c-gate, zh, t=(zsum-1)*c, u=zh_l-t, h_new=u+zh_r -> out."""
    C, nm, lp = st["C"], st["nm"], st["lp"]
    hs, z, zs, hc3 = st["hs"], st["z"], st["zs"], st["hc3"]
    bc = None if fuse else bias["bc"]

    hcand = lp["hc"].tile([128, 2, C], fp16, name=f"hc{nm}", tag="t")
    _gate(nc, P, Wt["uc"], hs[:, :, :], C,
          (lambda c0, n, m=None: hcand[:, :, c0:c0 + n] if m is None
           else hcand[:, m, c0:c0 + n]),
          AF.Tanh, bc, f"c{nm}")

    # zh = z * hc in place into z (DVE 2x)
    for c0 in range(0, 2 * C, CH_V):
        n = min(CH_V, 2 * C - c0)
        nc.vector.tensor_tensor(z[:, :, c0:c0 + n], z[:, :, c0:c0 + n],
                                hc3[:, :, c0:c0 + n], ALU.mult)

    # zs -= 1 in place (DVE 4x), then t = zs * hcand in place into hcand
    for c0 in range(0, C, CH_V):
        n = min(CH_V, C - c0)
        nc.vector.tensor_scalar(zs[:, :, c0:c0 + n], zs[:, :, c0:c0 + n],
                                -1.0, None, ALU.add)
    for c0 in range(0, C, CH_V):
        n = min(CH_V, C - c0)
        nc.vector.tensor_tensor(hcand[:, :, c0:c0 + n], zs[:, :, c0:c0 + n],
                                hcand[:, :, c0:c0 + n], ALU.mult)

    # u = zh_l - t, in place into zh_l; h_new = u + zh_r -> out
    for c0 in range(0, C, CH_V):
        n = min(CH_V, C - c0)
        nc.vector.tensor_tensor(z[:, :, c0:c0 + n], z[:, :, c0:c0 + n],
                                hcand[:, :, c0:c0 + n], ALU.subtract)
    for c0 in range(0, C, out_chunk):
        n = min(out_chunk, C - c0)
        nc.vector.tensor_tensor(out_sl(c0, n), z[:, :, c0:c0 + n],
                                z[:, :, C + c0:C + c0 + n], ALU.add)


def _build(fuse):
    nc = bacc.Bacc("TRN2", debug=False)

    xT_d = nc.dram_tensor("xT", [H, TPC * NLEAF], fp16, kind="ExternalInput")
    wd = {}
    for gnm in ("w", "uz", "ur", "uc"):
        wd[gnm] = nc.dram_tensor(f"{gnm}T", [H, H], fp16, kind="ExternalInput")
    bd = {}
    if not fuse:
        for bnm in ("bw", "bz", "br", "bc"):
            bd[bnm] = nc.dram_tensor(bnm, [H, 1], f32, kind="ExternalInput")
    hout_d = nc.dram_tensor("h_out", [H, OUTCOLS], fp16, kind="ExternalOutput")

    with tile.TileContext(nc) as tc:
        from contextlib import ExitStack
        with ExitStack() as ctx:
            P = {}

            def pool(name, bufs):
                P[name] = ctx.enter_context(tc.tile_pool(name=name, bufs=bufs))

            pool("const", 1)
            pool("xg", 4)
            pool("h10", 2)
            pool("h9", 2)
            pool("h8", 2)
            pool("h7", 2)
            pool("jbA", 1)
            pool("jbB", 1)
            pool("rc", 2)
            LP = {}
            for lv in (9, 8, 7, 6):
                for t_ in ("z", "hs", "hc", "zs"):
                    pool(f"{t_}{lv}", 1)
                LP[lv] = {t_: P[f"{t_}{lv}"] for t_ in ("z", "hs", "hc", "zs")}
            for ch in ("A", "B"):
                for t_ in ("z", "hs", "hc", "zs"):
                    pool(f"{t_}J{ch}", 1)
                LP[ch] = {t_: P[f"{t_}J{ch}"]
                          for t_ in ("z", "hs", "hc", "zs")}
            P["ps"] = ctx.enter_context(
                tc.tile_pool(name="ps", bufs=2, space="PSUM"))

            cp = P["const"]
            Wt = {}
            for gnm in ("w", "uz", "ur", "uc"):
                wtile = cp.tile([128, 2, H], fp16, name=f"{gnm}T")
                Wt[gnm] = wtile
                for k in range(2):
                    nc.sync.dma_start(wtile[:, k, :],
                                      wd[gnm].ap()[k * 128:(k + 1) * 128, :])
            bias = {}
            if not fuse:
                for bnm in ("bw", "bz", "br", "bc"):
                    bias[bnm] = [cp.tile([128, 1], f32, name=f"{bnm}{m}")
                                 for m in range(2)]
                    for m in range(2):
                        nc.sync.dma_start(bias[bnm][m][:],
                                          bd[bnm].ap()[m * 128:(m + 1) * 128, :])
            bw2 = None if fuse else bias["bw"]

            # warm-up: dummy matmuls on the weight tiles keep the PE busy
            # through the HAM activity window while the x DMA streams in, and
            # dummy activations pull the ACT table loads into the DMA shadow.
            scr = cp.tile([128, 2, 8], fp16, name="warm_scr")
            for r in range(2):
                wps = P["ps"].tile([128, 2, CH_B], f32, name=f"warm{r}",
                                   tag="ps")
                for m in range(2):
                    for k in range(2):
                        nc.tensor.matmul(wps[:, m, :512],
                                         Wt["w"][:, k, m * 128:(m + 1) * 128],
                                         Wt["uz"][:, :, :],
                                         start=(k == 0), stop=(k == 1))
                        nc.tensor.matmul(wps[:, m, 512:1024],
                                         Wt["ur"][:, k, m * 128:(m + 1) * 128],
                                         Wt["uc"][:, :, :],
                                         start=(k == 0), stop=(k == 1))
                if r == 0:
                    nc.scalar.activation(scr[:, 0, :], wps[:, 0, :8],
                                         AF.Sigmoid)
                    nc.scalar.activation(scr[:, 1, :], wps[:, 1, :8],
                                         AF.Tanh)

            jb = {"A": P["jbA"].tile([128, 2, JN * TPH], fp16, name="jbA"),
                  "B": P["jbB"].tile([128, 2, JN * TPH], fp16, name="jbB")}

            def jview(ch, l):
                a = JOFF[l] * TPH
                return jb[ch][:, :, a:a + (1 << l) * TPH]

            def load_x(g):
                ts = []
                for q in range(2):
                    xt = P["xg"].tile([128, 2, 2048], fp16,
                                      name=f"x{g}_{q}", tag="xg")
                    c0 = g * G * NLEAF + q * 2048
                    for cc in (0, 1024):
                        for k in range(2):
                            nc.sync.dma_start(
                                xt[:, k, cc:cc + 1024],
                                xT_d.ap()[k * 128:(k + 1) * 128,
                                          c0 + cc:c0 + cc + 1024])
                    ts.append(xt)
                return ts

            def emit_leaf(g, xts):
                h10 = P["h10"].tile([128, 2, G * NLEAF], fp16,
                                    name=f"h10g{g}", tag="h10")
                for q in range(2):
                    _gate(nc, P, Wt["w"], xts[q][:, :, :], 2048,
                          (lambda c0, n, m=None, q=q:
                           h10[:, :, q * 2048 + c0:q * 2048 + c0 + n] if m is None
                           else h10[:, m, q * 2048 + c0:q * 2048 + c0 + n]),
                          AF.Tanh, bw2, f"x{g}_{q}")
                for k in range(2):
                    nc.sync.dma_start(
                        hout_d.ap()[k * 128:(k + 1) * 128,
                                    OFF10 + g * 4096:OFF10 + (g + 1) * 4096],
                        h10[:, k, :])
                return h10

            def back_glevel(g, lv, st):
                C = st["C"]
                if lv == 6:
                    ch = "A" if g < 2 else "B"
                    off = (g % 2) * G
                    jv6 = jview(ch, 6)
                    out4 = jv6.rearrange("p a (s t) -> p a s t", t=TPH)
                    osl = (lambda c0, n: out4[:, :, :, off:off + G])
                    _level_back(nc, P, st, osl, C, Wt, bias, fuse)
                    return None
                pool_ = {9: "h9", 8: "h8", 7: "h7"}[lv]
                hnew = P[pool_].tile([128, 2, C], fp16,
                                     name=f"h{lv}g{g}", tag=pool_)
                _level_back(nc, P, st,
                            (lambda c0, n: hnew[:, :, c0:c0 + n]),
                            CH_B, Wt, bias, fuse)
                off = {9: OFF9, 8: OFF8, 7: OFF7}[lv]
                for k in range(2):
                    nc.sync.dma_start(
                        hout_d.ap()[k * 128:(k + 1) * 128,
                                    off + g * C:off + (g + 1) * C],
                        hnew[:, k, :])
                return hnew

            # wavefront with split-phase emission per tick
            gstate = {}
            xtiles = {0: load_x(0)}
            for t in range(14):
                items = []
                for g in range(NG):
                    s = t - g
                    if 1 <= s <= 4:
                        items.append(("g", g, 10 - s))
                if 6 <= t <= 11:
                    items.append(("c", "A", 11 - t))
                if 8 <= t <= 13:
                    items.append(("c", "B", 13 - t))

                sts = []
                for kind, gg, lv in items:
                    if kind == "g":
                        C = G * (1 << lv)
                        st = _level_front(nc, P, f"g{gg}l{lv}", C, LP[lv],
                                          gstate[gg][:, :, :], Wt, bias, fuse)
                    else:
                        C = TPH * (1 << lv)
                        st = _level_front(nc, P, f"j{gg}{lv}", C, LP[gg],
                                          jview(gg, lv + 1), Wt, bias, fuse)
                    sts.append(st)

                for (kind, gg, lv), st in zip(items, sts):
                    if kind == "g":
                        gstate[gg] = back_glevel(gg, lv, st)
                    else:
                        ov = jview(gg, lv)
                        _level_back(nc, P, st,
                                    (lambda c0, n, ov=ov: ov[:, :, c0:c0 + n]),
                                    CH_B, Wt, bias, fuse)

                for g in range(NG):
                    if t - g == 0:
                        if g + 1 < NG:
                            xtiles[g + 1] = load_x(g + 1)
                        gstate[g] = emit_leaf(g, xtiles.pop(g))

            # stream chain buffers out
            for ci, ch in enumerate(("A", "B")):
                a = OFFJ + ci * JN * TPH
                for k in range(2):
                    nc.sync.dma_start(
                        hout_d.ap()[k * 128:(k + 1) * 128, a:a + JN * TPH],
                        jb[ch][:, k, :])

    nc.compile()
    return nc


_NC = {}


def _get_nc(fuse=True):
    if fuse not in _NC:
        _NC[fuse] = _build(fuse)
    return _NC[fuse]


def make_in_maps(inputs, fuse):
    x = np.asarray(inputs["x"], np.float32)
    shared = {
        "wT": np.ascontiguousarray(np.asarray(inputs["W"], np.float32).T,
                                   dtype=np.float16),
        "uzT": np.ascontiguousarray(np.asarray(inputs["Uz"], np.float32).T,
                                    dtype=np.float16),
        "urT": np.ascontiguousarray(np.asarray(inputs["Ur"], np.float32).T,
                                    dtype=np.float16),
        "ucT": np.ascontiguousarray(np.asarray(inputs["Uc"], np.float32).T,
                                    dtype=np.float16),
    }
    if not fuse:
        shared.update({
            "bw": np.asarray(inputs["bW"], np.float32).reshape(H, 1),
            "bz": np.asarray(inputs["bz"], np.float32).reshape(H, 1),
            "br": np.asarray(inputs["br"], np.float32).reshape(H, 1),
            "bc": np.asarray(inputs["bc"], np.float32).reshape(H, 1),
        })
    sig10 = SIG[10]
    in_maps = []
    for c in range(NCORES):
        xc = x[c * TPC:(c + 1) * TPC, NLEAF - 1:, :]       # [16, 1024, 256]
        xs = xc[:, sig10, :]                                # slot order
        xT = xs.reshape(NG, G, NLEAF, H).transpose(3, 0, 2, 1).reshape(
            H, TPC * NLEAF)
        in_maps.append({"xT": np.ascontiguousarray(xT, dtype=np.float16),
                        **shared})
    return in_maps


def assemble_out(core_outs):
    out = np.empty((T, NN, H), np.float32)
    for c in range(NCORES):
        ho = np.asarray(core_outs[c])                       # [256, 32752] fp16
        oc = out[c * TPC:(c + 1) * TPC]
        for lv, off in ((10, OFF10), (9, OFF9), (8, OFF8), (7, OFF7)):
            Pl = 1 << lv
            blk = ho[:, off:off + TPC * Pl].reshape(H, NG, Pl, G)
            b = blk.transpose(1, 3, 2, 0).reshape(TPC, Pl, H)
            oc[:, (Pl - 1) + SIG[lv], :] = b.astype(np.float32)
        for ci in range(NCH):
            tc0 = ci * TPH
            for lv in range(6, -1, -1):
                Pl = 1 << lv
                a = OFFJ + ci * JN * TPH + JOFF[lv] * TPH
                blk = ho[:, a:a + Pl * TPH].reshape(H, Pl, TPH)
                oc[tc0:tc0 + TPH, (Pl - 1) + SIG[lv], :] = blk.transpose(
                    2, 1, 0).astype(np.float32)
    return out


def kernel(**inputs):
    assert int(inputs["depth"]) == DEPTH
    fuse = all(not np.any(np.asarray(inputs[b]))
               for b in ("bW", "br", "bc", "bz"))
    nc = _get_nc(fuse)
    in_maps = make_in_maps(inputs, fuse)
    res = run_bass_kernel_spmd(nc, in_maps, list(range(NCORES)))
    return assemble_out([r["h_out"] for r in res.results])


# revision 12
# speedup vs baseline: 1.2712x; 1.0060x over previous
"""ChildSum TreeGRU on 8 Trainium2 NeuronCores.

Data-parallel over trees (16/core). fp16 on-chip; feature dim folded as
[128 partitions, 2 k-halves, cols]; within each tree level, nodes are stored
in bit-reversed order so children of the (bit-reversed-ordered) parents form
two contiguous blocks [left | right] -> every DVE op is packed/contiguous and
runs in the 2-byte 2x mode.

v3: split-phase emission — each tick emits the front half (hs, z-gate,
r-gate+rh) of every active step before any back half (c-gate + tail), so
other steps' matmuls fill the ACT->rh latency window ahead of each c-gate
and the PE never idles long enough for the HAM clock gate to re-throttle.
h_new = (zh_l - (zsum-1)*c) + zh_r with a fused scalar_tensor_tensor.
Levels with 3C <= 1024 put z|r in one PSUM tile and run a single merged
sigmoid ACT (one less chain hop). 1024-col PSUM blocks (4 banks) halve the
ACT instruction count. The level<=5 phase runs as two independent 8-tree
chains overlapped with the group-phase wind-down.

Levels 10(leaf)..6 run per group of 4 trees (wavefront across 4 groups).
Host pre-permutes x and post-permutes the output (bit-reversal + layout),
which is free w.r.t. HW exec time.
"""
import sys

for p in ("/opt/trn_rl_repo", "/root/.axon_site/_ro/trn_rl_repo"):
    if p not in sys.path:
        sys.path.insert(0, p)

import numpy as np
import concourse.tile as tile
from concourse import bacc, mybir
from concourse.bass_utils import run_bass_kernel_spmd

f32 = mybir.dt.float32
fp16 = mybir.dt.float16
AF = mybir.ActivationFunctionType
ALU = mybir.AluOpType

T, DEPTH, NN, H = 128, 11, 2047, 256
NCORES = 8
TPC = T // NCORES          # 16 trees per core
G = 4                      # trees per group
NG = TPC // G              # 4 groups
NLEAF = 1 << (DEPTH - 1)   # 1024

CH_MM = 512                # matmul chunk (one psum bank per (m, chunk))
CH_B = 1024                # psum block cols (4 banks; one ACT per block)
CH_V = 2048                # DVE chunk cols

# h_out column blocks (device layout, fp16). Group levels 10..7 then chains.
OFF10 = 0
OFF9 = OFF10 + TPC * 1024
OFF8 = OFF9 + TPC * 512
OFF7 = OFF8 + TPC * 256
OFFJ = OFF7 + TPC * 128            # 30720
JN = 127                            # nodes/tree in chain buf (levels 0..6)
NCH = 2                             # chains (8 trees each)
TPH = TPC // NCH                    # 8 trees per chain
OUTCOLS = OFFJ + NCH * JN * TPH     # 32752
# chain-buf block start (in slots) for level l: levels 6,5,...,0 high->low
JOFF = {l: (JN - ((1 << (l + 1)) - 1)) for l in range(7)}


def _sig(l):
    s = np.zeros(1, np.int64)
    for _ in range(l):
        s = np.concatenate([2 * s, 2 * s + 1])
    return s


SIG = {l: _sig(l) for l in range(DEPTH)}


def _gate(nc, P, wt3, rhs3, cols, out_sl, func, bias2, nm, rh_into=None):
    """out = func(U @ rhs + b) over `cols` columns, in 1024-col psum blocks."""
    for c0 in range(0, cols, CH_B):
        n = min(CH_B, cols - c0)
        ps = P["ps"].tile([128, 2, CH_B], f32, name=f"ps{nm}_{c0}", tag="ps")
        for m in range(2):
            for k in range(2):
                for cc in range(0, n, CH_MM):
                    w = min(CH_MM, n - cc)
                    nc.tensor.matmul(ps[:, m, cc:cc + w],
                                     wt3[:, k, m * 128:(m + 1) * 128],
                                     rhs3[:, k, c0 + cc:c0 + cc + w],
                                     start=(k == 0), stop=(k == 1))
        if rh_into is None:
            if bias2 is None:
                nc.scalar.activation(out_sl(c0, n), ps[:, :, :n], func)
            else:
                for m in range(2):
                    nc.scalar.activation(out_sl(c0, n, m), ps[:, m, :n], func,
                                         bias=bias2[m][:])
        else:
            rc = P["rc"].tile([128, 2, CH_B], fp16, name=f"r{nm}_{c0}",
                              tag="rc")
            if bias2 is None:
                nc.scalar.activation(rc[:, :, :n], ps[:, :, :n], func)
            else:
                for m in range(2):
                    nc.scalar.activation(rc[:, m, :n], ps[:, m, :n], func,
                                         bias=bias2[m][:])
            nc.vector.tensor_tensor(rh_into[:, :, c0:c0 + n], rc[:, :, :n],
                                    rh_into[:, :, c0:c0 + n], ALU.mult)


def _mm_cols(nc, ps, wt3, rhs3, cols, p0):
    """Matmuls of wt3 @ rhs3 (all `cols`) into psum block at col offset p0."""
    for m in range(2):
        for k in range(2):
            for cc in range(0, cols, CH_MM):
                w = min(CH_MM, cols - cc)
                nc.tensor.matmul(ps[:, m, p0 + cc:p0 + cc + w],
                                 wt3[:, k, m * 128:(m + 1) * 128],
                                 rhs3[:, k, cc:cc + w],
                                 start=(k == 0), stop=(k == 1))


def _level_front(nc, P, nm, C, lp, hc3, Wt, bias, fuse):
    """hs, z-gate, r-gate (+rh into hs), zsum. Returns state for the back."""
    bz = None if fuse else bias["bz"]
    br = None if fuse else bias["br"]

    hs = lp["hs"].tile([128, 2, C], fp16, name=f"hs{nm}", tag="t")
    for c0 in range(0, C, CH_V):
        n = min(CH_V, C - c0)
        nc.vector.tensor_tensor(hs[:, :, c0:c0 + n], hc3[:, :, c0:c0 + n],
                                hc3[:, :, C + c0:C + c0 + n], ALU.add)

    if 3 * C <= CH_B and fuse:
        # merged z|r sigmoid: one psum block [z(2C) | r(C)], one ACT
        zr = lp["z"].tile([128, 2, 3 * C], fp16, name=f"zr{nm}", tag="t")
        ps = P["ps"].tile([128, 2, CH_B], f32, name=f"pzr{nm}", tag="ps")
        _mm_cols(nc, ps, Wt["uz"], hc3, 2 * C, 0)
        _mm_cols(nc, ps, Wt["ur"], hs, C, 2 * C)
        nc.scalar.activation(zr[:, :, :], ps[:, :, :3 * C], AF.Sigmoid)
        z = zr[:, :, :2 * C]
        nc.vector.tensor_tensor(hs[:, :, :], zr[:, :, 2 * C:3 * C],
                                hs[:, :, :], ALU.mult)
    else:
        zt = lp["z"].tile([128, 2, 2 * C], fp16, name=f"z{nm}", tag="t")
        z = zt[:, :, :]
        _gate(nc, P, Wt["uz"], hc3, 2 * C,
              (lambda c0, n, m=None: zt[:, :, c0:c0 + n] if m is None
               else zt[:, m, c0:c0 + n]),
              AF.Sigmoid, bz, f"z{nm}")
        _gate(nc, P, Wt["ur"], hs[:, :, :], C, None, AF.Sigmoid, br, f"r{nm}",
              rh_into=hs)

    zs = lp["zs"].tile([128, 2, C], fp16, name=f"zs{nm}", tag="t")
    c0 = 0
    if C >= 1024:
        # gpsimd takes the first half in 512-col chunks (proven size)
        for c0 in range(0, C // 2, 512):
            nc.gpsimd.tensor_tensor(zs[:, :, c0:c0 + 512],
                                    z[:, :, c0:c0 + 512],
                                    z[:, :, C + c0:C + c0 + 512], ALU.add)
        c0 = C // 2
    for c0 in range(c0, C, CH_V):
        n = min(CH_V, C - c0)
        nc.vector.tensor_tensor(zs[:, :, c0:c0 + n], z[:, :, c0:c0 + n],
                                z[:, :, C + c0:C + c0 + n], ALU.add)

    return {"hs": hs, "z": z, "zs": zs, "hc3": hc3, "C": C, "nm": nm,
            "lp": lp}


def _level_back(nc, P, st, out_sl, out_chunk, Wt, bias, fuse):
    """c-gate, zh, t=(zsum-1)*c, u=zh_l-t, h_new=u+zh_r -> out."""
    C, nm, lp = st["C"], st["nm"], st["lp"]
    hs, z, zs, hc3 = st["hs"], st["z"], st["zs"], st["hc3"]
    bc = None if fuse else bias["bc"]

    hcand = lp["hc"].tile([128, 2, C], fp16, name=f"hc{nm}", tag="t")
    _gate(nc, P, Wt["uc"], hs[:, :, :], C,
          (lambda c0, n, m=None: hcand[:, :, c0:c0 + n] if m is None
           else hcand[:, m, c0:c0 + n]),
          AF.Tanh, bc, f"c{nm}")

    # zs -= 1 in place (DVE 4x); zh = z * hc in place into z (DVE 2x);
    # v = zh_l + zh_r in place into zh_l — all run during c-MM/ACT
    for c0 in range(0, C, CH_V):
        n = min(CH_V, C - c0)
        nc.vector.tensor_scalar(zs[:, :, c0:c0 + n], zs[:, :, c0:c0 + n],
                                -1.0, None, ALU.add)
    for c0 in range(0, 2 * C, CH_V):
        n = min(CH_V, 2 * C - c0)
        nc.vector.tensor_tensor(z[:, :, c0:c0 + n], z[:, :, c0:c0 + n],
                                hc3[:, :, c0:c0 + n], ALU.mult)
    for c0 in range(0, C, CH_V):
        n = min(CH_V, C - c0)
        nc.vector.tensor_tensor(z[:, :, c0:c0 + n], z[:, :, c0:c0 + n],
                                z[:, :, C + c0:C + c0 + n], ALU.add)

    # post-c-ACT tail (2 hops): t = zs * hcand; h_new = v - t -> out
    for c0 in range(0, C, CH_V):
        n = min(CH_V, C - c0)
        nc.vector.tensor_tensor(hcand[:, :, c0:c0 + n], zs[:, :, c0:c0 + n],
                                hcand[:, :, c0:c0 + n], ALU.mult)
    for c0 in range(0, C, out_chunk):
        n = min(out_chunk, C - c0)
        nc.vector.tensor_tensor(out_sl(c0, n), z[:, :, c0:c0 + n],
                                hcand[:, :, c0:c0 + n], ALU.subtract)


def _build(fuse):
    nc = bacc.Bacc("TRN2", debug=False)

    xT_d = nc.dram_tensor("xT", [H, TPC * NLEAF], fp16, kind="ExternalInput")
    wd = {}
    for gnm in ("w", "uz", "ur", "uc"):
        wd[gnm] = nc.dram_tensor(f"{gnm}T", [H, H], fp16, kind="ExternalInput")
    bd = {}
    if not fuse:
        for bnm in ("bw", "bz", "br", "bc"):
            bd[bnm] = nc.dram_tensor(bnm, [H, 1], f32, kind="ExternalInput")
    hout_d = nc.dram_tensor("h_out", [H, OUTCOLS], fp16, kind="ExternalOutput")

    with tile.TileContext(nc) as tc:
        from contextlib import ExitStack
        with ExitStack() as ctx:
            P = {}

            def pool(name, bufs):
                P[name] = ctx.enter_context(tc.tile_pool(name=name, bufs=bufs))

            pool("const", 1)
            pool("xg", 4)
            pool("h10", 2)
            pool("h9", 2)
            pool("h8", 2)
            pool("h7", 2)
            pool("jbA", 1)
            pool("jbB", 1)
            pool("rc", 2)
            LP = {}
            for lv in (9, 8, 7, 6):
                for t_ in ("z", "hs", "hc", "zs"):
                    pool(f"{t_}{lv}", 1)
                LP[lv] = {t_: P[f"{t_}{lv}"] for t_ in ("z", "hs", "hc", "zs")}
            for ch in ("A", "B"):
                for t_ in ("z", "hs", "hc", "zs"):
                    pool(f"{t_}J{ch}", 1)
                LP[ch] = {t_: P[f"{t_}J{ch}"]
                          for t_ in ("z", "hs", "hc", "zs")}
            P["ps"] = ctx.enter_context(
                tc.tile_pool(name="ps", bufs=2, space="PSUM"))

            cp = P["const"]
            Wt = {}
            for gnm in ("w", "uz", "ur", "uc"):
                wtile = cp.tile([128, 2, H], fp16, name=f"{gnm}T")
                Wt[gnm] = wtile
                nc.gpsimd.dma_start(
                    wtile[:, :, :],
                    wd[gnm].ap().rearrange("(a p) h -> p a h", a=2))
            bias = {}
            if not fuse:
                for bnm in ("bw", "bz", "br", "bc"):
                    bias[bnm] = [cp.tile([128, 1], f32, name=f"{bnm}{m}")
                                 for m in range(2)]
                    for m in range(2):
                        nc.gpsimd.dma_start(bias[bnm][m][:],
                                            bd[bnm].ap()[m * 128:(m + 1) * 128, :])
            bw2 = None if fuse else bias["bw"]

            # warm-up: dummy matmuls on the weight tiles keep the PE busy
            # through the HAM activity window while the x DMA streams in, and
            # dummy activations pull the ACT table loads into the DMA shadow.
            scr = cp.tile([128, 2, 8], fp16, name="warm_scr")
            for r in range(2):
                wps = P["ps"].tile([128, 2, CH_B], f32, name=f"warm{r}",
                                   tag="ps")
                for m in range(2):
                    for k in range(2):
                        nc.tensor.matmul(wps[:, m, :512],
                                         Wt["w"][:, k, m * 128:(m + 1) * 128],
                                         Wt["uz"][:, :, :],
                                         start=(k == 0), stop=(k == 1))
                        nc.tensor.matmul(wps[:, m, 512:1024],
                                         Wt["ur"][:, k, m * 128:(m + 1) * 128],
                                         Wt["uc"][:, :, :],
                                         start=(k == 0), stop=(k == 1))
                if r == 0:
                    nc.scalar.activation(scr[:, 0, :], wps[:, 0, :8],
                                         AF.Sigmoid)
                    nc.scalar.activation(scr[:, 1, :], wps[:, 1, :8],
                                         AF.Tanh)

            jb = {"A": P["jbA"].tile([128, 2, JN * TPH], fp16, name="jbA"),
                  "B": P["jbB"].tile([128, 2, JN * TPH], fp16, name="jbB")}

            def jview(ch, l):
                a = JOFF[l] * TPH
                return jb[ch][:, :, a:a + (1 << l) * TPH]

            def load_x(g):
                ts = []
                for q in range(2):
                    xt = P["xg"].tile([128, 2, 2048], fp16,
                                      name=f"x{g}_{q}", tag="xg")
                    c0 = g * G * NLEAF + q * 2048
                    for cc in (0, 1024):
                        nc.sync.dma_start(
                            xt[:, :, cc:cc + 1024],
                            xT_d.ap()[:, c0 + cc:c0 + cc + 1024].rearrange(
                                "(a p) c -> p a c", a=2))
                    ts.append(xt)
                return ts

            def emit_leaf(g, xts):
                h10 = P["h10"].tile([128, 2, G * NLEAF], fp16,
                                    name=f"h10g{g}", tag="h10")
                for q in range(2):
                    _gate(nc, P, Wt["w"], xts[q][:, :, :], 2048,
                          (lambda c0, n, m=None, q=q:
                           h10[:, :, q * 2048 + c0:q * 2048 + c0 + n] if m is None
                           else h10[:, m, q * 2048 + c0:q * 2048 + c0 + n]),
                          AF.Tanh, bw2, f"x{g}_{q}")
                nc.sync.dma_start(
                    hout_d.ap()[:, OFF10 + g * 4096:OFF10 + (g + 1) * 4096]
                    .rearrange("(a p) c -> p a c", a=2), h10[:, :, :])
                return h10

            def back_glevel(g, lv, st):
                C = st["C"]
                if lv == 6:
                    ch = "A" if g < 2 else "B"
                    off = (g % 2) * G
                    jv6 = jview(ch, 6)
                    out4 = jv6.rearrange("p a (s t) -> p a s t", t=TPH)
                    osl = (lambda c0, n: out4[:, :, :, off:off + G])
                    _level_back(nc, P, st, osl, C, Wt, bias, fuse)
                    return None
                pool_ = {9: "h9", 8: "h8", 7: "h7"}[lv]
                hnew = P[pool_].tile([128, 2, C], fp16,
                                     name=f"h{lv}g{g}", tag=pool_)
                _level_back(nc, P, st,
                            (lambda c0, n: hnew[:, :, c0:c0 + n]),
                            CH_B, Wt, bias, fuse)
                off = {9: OFF9, 8: OFF8, 7: OFF7}[lv]
                nc.sync.dma_start(
                    hout_d.ap()[:, off + g * C:off + (g + 1) * C]
                    .rearrange("(a p) c -> p a c", a=2), hnew[:, :, :])
                return hnew

            # wavefront with split-phase emission per tick
            gstate = {}
            xtiles = {0: load_x(0)}
            for t in range(14):
                items = []
                for g in range(NG):
                    s = t - g
                    if 1 <= s <= 4:
                        items.append(("g", g, 10 - s))
                if 6 <= t <= 11:
                    items.append(("c", "A", 11 - t))
                if 8 <= t <= 13:
                    items.append(("c", "B", 13 - t))

                sts = []
                for kind, gg, lv in items:
                    if kind == "g":
                        C = G * (1 << lv)
                        st = _level_front(nc, P, f"g{gg}l{lv}", C, LP[lv],
                                          gstate[gg][:, :, :], Wt, bias, fuse)
                    else:
                        C = TPH * (1 << lv)
                        st = _level_front(nc, P, f"j{gg}{lv}", C, LP[gg],
                                          jview(gg, lv + 1), Wt, bias, fuse)
                    sts.append(st)

                def emit_leaves():
                    for g in range(NG):
                        if t - g == 0:
                            if g + 1 < NG:
                                xtiles[g + 1] = load_x(g + 1)
                            gstate[g] = emit_leaf(g, xtiles.pop(g))

                if len(items) <= 1:
                    emit_leaves()

                for (kind, gg, lv), st in zip(items, sts):
                    if kind == "g":
                        gstate[gg] = back_glevel(gg, lv, st)
                    else:
                        ov = jview(gg, lv)
                        _level_back(nc, P, st,
                                    (lambda c0, n, ov=ov: ov[:, :, c0:c0 + n]),
                                    CH_B, Wt, bias, fuse)

                if len(items) > 1:
                    emit_leaves()

            # stream chain buffers out
            for ci, ch in enumerate(("A", "B")):
                a = OFFJ + ci * JN * TPH
                nc.sync.dma_start(
                    hout_d.ap()[:, a:a + JN * TPH]
                    .rearrange("(a p) c -> p a c", a=2), jb[ch][:, :, :])

    nc.compile()
    return nc


_NC = {}


def _get_nc(fuse=True):
    if fuse not in _NC:
        _NC[fuse] = _build(fuse)
    return _NC[fuse]


def make_in_maps(inputs, fuse):
    x = np.asarray(inputs["x"], np.float32)
    shared = {
        "wT": np.ascontiguousarray(np.asarray(inputs["W"], np.float32).T,
                                   dtype=np.float16),
        "uzT": np.ascontiguousarray(np.asarray(inputs["Uz"], np.float32).T,
                                    dtype=np.float16),
        "urT": np.ascontiguousarray(np.asarray(inputs["Ur"], np.float32).T,
                                    dtype=np.float16),
        "ucT": np.ascontiguousarray(np.asarray(inputs["Uc"], np.float32).T,
                                    dtype=np.float16),
    }
    if not fuse:
        shared.update({
            "bw": np.asarray(inputs["bW"], np.float32).reshape(H, 1),
            "bz": np.asarray(inputs["bz"], np.float32).reshape(H, 1),
            "br": np.asarray(inputs["br"], np.float32).reshape(H, 1),
            "bc": np.asarray(inputs["bc"], np.float32).reshape(H, 1),
        })
    sig10 = SIG[10]
    in_maps = []
    for c in range(NCORES):
        xc = x[c * TPC:(c + 1) * TPC, NLEAF - 1:, :]       # [16, 1024, 256]
        xs = xc[:, sig10, :]                                # slot order
        xT = xs.reshape(NG, G, NLEAF, H).transpose(3, 0, 2, 1).reshape(
            H, TPC * NLEAF)
        in_maps.append({"xT": np.ascontiguousarray(xT, dtype=np.float16),
                        **shared})
    return in_maps


def assemble_out(core_outs):
    out = np.empty((T, NN, H), np.float32)
    for c in range(NCORES):
        ho = np.asarray(core_outs[c])                       # [256, 32752] fp16
        oc = out[c * TPC:(c + 1) * TPC]
        for lv, off in ((10, OFF10), (9, OFF9), (8, OFF8), (7, OFF7)):
            Pl = 1 << lv
            blk = ho[:, off:off + TPC * Pl].reshape(H, NG, Pl, G)
            b = blk.transpose(1, 3, 2, 0).reshape(TPC, Pl, H)
            oc[:, (Pl - 1) + SIG[lv], :] = b.astype(np.float32)
        for ci in range(NCH):
            tc0 = ci * TPH
            for lv in range(6, -1, -1):
                Pl = 1 << lv
                a = OFFJ + ci * JN * TPH + JOFF[lv] * TPH
                blk = ho[:, a:a + Pl * TPH].reshape(H, Pl, TPH)
                oc[tc0:tc0 + TPH, (Pl - 1) + SIG[lv], :] = blk.transpose(
                    2, 1, 0).astype(np.float32)
    return out


def kernel(**inputs):
    assert int(inputs["depth"]) == DEPTH
    fuse = all(not np.any(np.asarray(inputs[b]))
               for b in ("bW", "br", "bc", "bz"))
    nc = _get_nc(fuse)
    in_maps = make_in_maps(inputs, fuse)
    res = run_bass_kernel_spmd(nc, in_maps, list(range(NCORES)))
    return assemble_out([r["h_out"] for r in res.results])
